# revision 20
# baseline (speedup 1.0000x reference)
"""Trainium2 Bass kernel for nn_MemoryEfficientRGBDecoderStem.

Reference computation (per batch b of 16):
  - Fourier positional embedding: 32 channels of cos((grid - pos)*std*pi/2*2^f),
    separable in x / y.
  - inp = concat([gestalt (256ch) * mask, embedding (32ch) * mask, depth * mask])
  - space-to-depth 2x2 patchify -> rows (64*64 patches, 289*4=1156 cols)
  - h = silu(p @ W1.T + b1)  (hidden 1024)
  - o = h @ W2.T + b2        (base 256) -> (B, 256, 64, 64)

Strategy: data-parallel over batch, 2 batches per NeuronCore (8 cores).
The GEMM1 contraction is algebraically reduced from K=1156 to K=136:
  * the 1024 gestalt*mask columns collapse to 4 "mask" features with
    per-batch folded weights wg[s,n] = sum_c gestalt[c] * W1[n,(c,s)]
    (wg computed on-device with a small matmul),
  * the 128 cos-embedding columns are built on device via one-hot
    broadcast matmuls (cos tables x one-hot over px/py) times mask,
  * 4 depth*mask features.
All matmuls run in float32r (fast fp32 mode, 1 cycle/row at N>=512).
"""

import numpy as np

# ---- static problem shapes ----
B, G, H, W = 16, 256, 128, 128
NF, S = 16, 2
HID, BASE = 1024, 256
NCORES = 8
BPC = B // NCORES          # batches per core = 2
PH, PW = H // S, W // S    # 64, 64
R = PH * PW                # 4096 patch rows per batch
CH = 512                   # r-chunk size
NCH = R // CH              # 8 chunks per batch
NT1 = HID // 128           # 8 n-tiles for GEMM1
NT2 = BASE // 128          # 2 m-tiles for GEMM2
KT2 = HID // 128           # 8 k-tiles for GEMM2

_cache = {}


def _build_nc():
    import concourse.tile as tile
    from concourse import bacc, mybir

    dt = mybir.dt
    f32, f32r = dt.float32, dt.float32r

    nc = bacc.Bacc("TRN2", target_bir_lowering=False, debug=False,
                   num_devices=NCORES)

    # ---- DRAM I/O (per core) ----
    d_mdf = nc.dram_tensor("mdf", [BPC, 8, R], f32, kind="ExternalInput").ap()
    d_cxcy = nc.dram_tensor("cxcy", [72, BPC * NCH * 128], f32,
                            kind="ExternalInput").ap()
    d_ohxy = nc.dram_tensor("ohxy", [72, CH], f32, kind="ExternalInput").ap()
    d_selm = nc.dram_tensor("selm", [4, 128], f32, kind="ExternalInput").ap()
    d_w1a = nc.dram_tensor("w1a", [128, HID], f32, kind="ExternalInput").ap()
    d_w1bd = nc.dram_tensor("w1bd", [4, HID], f32, kind="ExternalInput").ap()
    d_w1g = nc.dram_tensor("w1g", [128, 2 * 4 * HID], f32, kind="ExternalInput").ap()
    d_gest = nc.dram_tensor("gest2", [128, 2 * BPC], f32, kind="ExternalInput").ap()
    d_w2s = nc.dram_tensor("w2s", [128, KT2 * BASE], f32, kind="ExternalInput").ap()
    d_b1t = nc.dram_tensor("b1t", [128, NT1], f32, kind="ExternalInput").ap()
    d_b2t = nc.dram_tensor("b2t", [128, NT2], f32, kind="ExternalInput").ap()
    d_out = nc.dram_tensor("out", [BPC, BASE, R], f32, kind="ExternalOutput").ap()

    with tile.TileContext(nc) as tc:
        with tc.tile_pool(name="const", bufs=1) as cp, \
             tc.tile_pool(name="stage", bufs=2) as stp, \
             tc.tile_pool(name="feat", bufs=3) as fp, \
             tc.tile_pool(name="h", bufs=2) as hp, \
             tc.tile_pool(name="outp", bufs=4) as op, \
             tc.tile_pool(name="ps_feat", bufs=1, space="PSUM") as psf, \
             tc.tile_pool(name="ps_g1", bufs=3, space="PSUM") as ps1, \
             tc.tile_pool(name="ps_g2", bufs=2, space="PSUM") as ps2:

            # ---- load + round constants ----
            def load_round(dram_ap, shape, pool=cp):
                st = stp.tile(shape, f32, tag="stage")
                nc.sync.dma_start(st[:], dram_ap)
                rt = pool.tile(shape, f32r, tag=f"r{dram_ap.tensor.name}")
                nc.vector.tensor_copy(rt[:], st[:])
                return rt

            w1a_r = load_round(d_w1a, [128, HID])
            w2s_r = load_round(d_w2s, [128, KT2 * BASE])
            cxcy_r = load_round(d_cxcy, [72, BPC * NCH * 128])
            ohxy_r = load_round(d_ohxy, [72, CH])
            selm_r = load_round(d_selm, [4, 128])

            b1t = cp.tile([128, NT1], f32)
            nc.sync.dma_start(b1t[:], d_b1t)
            b2t = cp.tile([128, NT2], f32)
            nc.sync.dma_start(b2t[:], d_b2t)

            blhsT = []
            # transient pools for the gestalt fold; close before the main loop
            with tc.tile_pool(name="wg1", bufs=1) as wg1, \
                 tc.tile_pool(name="wg2", bufs=2) as wg2, \
                 tc.tile_pool(name="ps_wg", bufs=1, space="PSUM") as pswg:
                gest_r = load_round(d_gest, [128, 2 * BPC], pool=wg1)
                # wg[b, s*HID+n] = sum_c gestalt[b,c] W1[n,(c,s)],
                # streaming w1g in 512-wide column chunks
                wgst = wg1.tile([BPC, 4 * HID], f32, tag="wgst")
                for sub in range(4 * HID // 512):
                    ps = pswg.tile([BPC, 512], f32, tag="wgps")
                    for kt in range(2):
                        wc = wg2.tile([128, 512], f32, tag="wgchunk")
                        nc.sync.dma_start(
                            wc[:],
                            d_w1g[:, kt * 4 * HID + sub * 512:
                                  kt * 4 * HID + (sub + 1) * 512])
                        wcr = wg2.tile([128, 512], f32r, tag="wgchunkr")
                        nc.vector.tensor_copy(wcr[:], wc[:])
                        nc.tensor.matmul(
                            ps[:], gest_r[:, kt * BPC:(kt + 1) * BPC], wcr[:],
                            start=(kt == 0), stop=(kt == 1))
                    nc.scalar.copy(wgst[:, sub * 512:(sub + 1) * 512], ps[:])

                # blhsT per batch: rows 0..3 = wg, rows 4..7 = depth W1.
                # Re-partition (1,4096) -> (4,1024) via a DRAM round-trip.
                with tc.tile_pool(name="wgd", bufs=1, space="DRAM") as wgd:
                    dwg = wgd.tile([BPC, 4, HID], f32, tag="dwg")
                    nc.sync.dma_start(dwg[:], wgst[:].rearrange(
                        "b (s n) -> b s n", s=4))
                    for b in range(BPC):
                        bl = wg1.tile([8, HID], f32, tag=f"blhsT{b}")
                        nc.sync.dma_start(bl[0:4, :], dwg[b])
                        nc.sync.dma_start(bl[4:8, :], d_w1bd)
                        blr = cp.tile([8, HID], f32r, tag=f"blhsTr{b}")
                        nc.vector.tensor_copy(blr[:], bl[:])
                        blhsT.append(blr)

            # ---- per-batch mask/depth features ----
            # mdf rows 0..3 = mask_s[r], rows 4..7 = depth_s[r] -> *= mask.
            # Compute engines can only address partition bases 0/32/64/96,
            # so build the depth*mask product at base 0 and DMA it into
            # partitions 4..7 of the rounded tile.
            mdf_r = []
            with tc.tile_pool(name="mdfstp", bufs=1) as mstp:
                for b in range(BPC):
                    mkf = mstp.tile([4, R], f32, tag=f"maskst{b}")
                    nc.sync.dma_start(mkf[:], d_mdf[b, 0:4])
                    dpf = mstp.tile([4, R], f32, tag="depthst")
                    nc.sync.dma_start(dpf[:], d_mdf[b, 4:8])
                    dpr = mstp.tile([4, R], f32r, tag="depthr")
                    nc.vector.tensor_tensor(dpr[:], dpf[:], mkf[:],
                                            mybir.AluOpType.mult)
                    mr = cp.tile([8, R], f32r, tag=f"mdfr{b}")
                    nc.vector.tensor_copy(mr[0:4, :], mkf[:])
                    nc.sync.dma_start(mr[4:8, :], dpr[:])
                    mdf_r.append(mr)

            # ---- main loop over (batch, r-chunk) ----
            for b in range(BPC):
                for cc in range(NCH):
                    r0 = cc * CH
                    sl = slice(r0, r0 + CH)

                    # mask broadcast to 128 partitions (k = f*4+s -> s = k%4)
                    mrep_ps = psf.tile([128, CH], f32, tag="mrep")
                    nc.tensor.matmul(mrep_ps[:], selm_r[:],
                                     mdf_r[b][0:4, sl], start=True, stop=True)
                    mrep = fp.tile([128, CH], f32, tag="mrep_sb")
                    nc.scalar.copy(mrep[:], mrep_ps[:])

                    # cos values broadcast over the chunk: single K=72 matmul
                    # with block-diagonal lhsT [cx (64 px rows) | cy (8 py
                    # rows)] against the stacked one-hot rhs (identical for
                    # every 512-r chunk).
                    blk = b * NCH + cc
                    cv_ps = psf.tile([128, CH], f32, tag="cv")
                    nc.tensor.matmul(cv_ps[:],
                                     cxcy_r[:, blk * 128:(blk + 1) * 128],
                                     ohxy_r[:], start=True, stop=True)

                    # embedding features = cos * mask (rounded to float32r)
                    fa = fp.tile([128, CH], f32r, tag="fa")
                    nc.vector.tensor_mul(fa[:], cv_ps[:], mrep[:])

                    # GEMM1 + bias1 + SiLU -> hT chunk (8 n-tiles side by side)
                    hT = hp.tile([128, NT1 * CH], f32r, tag="hT")
                    for nt in range(NT1):
                        g1 = ps1.tile([128, CH], f32, tag="g1")
                        nc.tensor.matmul(g1[:],
                                         w1a_r[:, nt * 128:(nt + 1) * 128],
                                         fa[:], start=True, stop=False)
                        nc.tensor.matmul(g1[:],
                                         blhsT[b][:, nt * 128:(nt + 1) * 128],
                                         mdf_r[b][:, sl], start=False, stop=True)
                        nc.scalar.activation(
                            hT[:, nt * CH:(nt + 1) * CH], g1[:],
                            mybir.ActivationFunctionType.Silu,
                            bias=b1t[:, nt:nt + 1], scale=1.0)

                    # GEMM2 + bias2 -> out chunk
                    for mt in range(NT2):
                        g2 = ps2.tile([128, CH], f32, tag="g2")
                        for kt in range(KT2):
                            nc.tensor.matmul(
                                g2[:],
                                w2s_r[:, kt * BASE + mt * 128:
                                      kt * BASE + (mt + 1) * 128],
                                hT[:, kt * CH:(kt + 1) * CH],
                                start=(kt == 0), stop=(kt == KT2 - 1))
                        ob = op.tile([128, CH], f32, tag="ob")
                        nc.vector.tensor_scalar_add(ob[:], g2[:],
                                                    b2t[:, mt:mt + 1])
                        nc.sync.dma_start(
                            d_out[b, mt * 128:(mt + 1) * 128, sl], ob[:])

    nc.compile()
    return nc


def _host_prep(position, gestalt, mask, depth, weight1, bias1, weight2, bias2):
    """Pure-layout host prep + tiny cos tables. Returns per-core input maps."""
    f32 = np.float32
    # --- cos tables, replicating the reference's fp32 op order exactly ---
    gx = np.arange(W, dtype=f32)
    gx = gx / f32(W - 1)
    gx = gx * f32(2)
    gx = gx - f32(1)          # (128,) == gy since H == W
    x = np.clip(position[:, 0], f32(-1.0), f32(1.0)).astype(f32)[:, None]
    y = np.clip(position[:, 1], f32(-1.0), f32(1.0)).astype(f32)[:, None]
    min_std = f32(1.0 / min(H, W))
    std = (f32(0.1) / np.clip(position[:, 3], min_std, f32(0.5)).astype(f32))[:, None]
    half_pi = f32(np.pi / 2)
    ngx = ((gx[None, :] - x) * std) * half_pi          # (B, 128) fp32
    ngy = ((gx[None, :] - y) * std) * half_pi
    freqs = (f32(2.0) ** np.arange(NF, dtype=f32))[None, :, None]
    ax = (ngx[:, None, :] * freqs).astype(f32)         # (B, NF, 128)
    ay = (ngy[:, None, :] * freqs).astype(f32)
    cosx = np.cos(ax.astype(np.float64)).astype(f32)   # true cos of fp32 angle
    cosy = np.cos(ay.astype(np.float64)).astype(f32)

    # cxa[b, px, f*4+s] = cosx[b, f, 2px + s%2]; cya uses s//2 with py
    cxa = np.empty((B, 64, 64), dtype=f32)
    cya = np.empty((B, 64, 64), dtype=f32)
    px2 = 2 * np.arange(64)
    for s in range(4):
        sx, sy = s % 2, s // 2
        cxa[:, :, np.arange(NF) * 4 + s] = cosx[:, :, px2 + sx].transpose(0, 2, 1)
        cya[:, :, np.arange(NF) * 4 + s] = cosy[:, :, px2 + sy].transpose(0, 2, 1)

    # --- mask/depth patchify: mdf[b, 0:4, r]=mask_s, [4:8]=depth_s (raw) ---
    def patch(a):  # (B,1,H,W) -> (B, 4, R) with s=(sy*2+sx), r=py*64+px
        a6 = a[:, 0].reshape(B, PH, S, PW, S)
        return a6.transpose(0, 2, 4, 1, 3).reshape(B, 4, R).astype(f32)

    mdf = np.concatenate([patch(mask), patch(depth)], axis=1)  # (B, 8, R)

    # --- weight reshuffles ---
    w1e = weight1[:, G * 4:(G + 2 * NF) * 4].reshape(HID, 2 * NF, 4)
    w1a = np.concatenate([
        w1e[:, 0::2, :].transpose(1, 2, 0).reshape(64, HID),   # even ch (cos_x)
        w1e[:, 1::2, :].transpose(1, 2, 0).reshape(64, HID),   # odd ch (cos_y)
    ], axis=0).astype(f32)                                     # (128, HID)
    w1bd = weight1[:, (G + 2 * NF) * 4:].T.astype(f32).copy()  # (4, HID)
    w1g = weight1[:, :G * 4].reshape(HID, G, 4).transpose(1, 2, 0)  # (256,4,HID)
    w1g = w1g.reshape(2, 128, 4 * HID).transpose(1, 0, 2).reshape(128, 2 * 4 * HID)
    w1g = np.ascontiguousarray(w1g, dtype=f32)
    w2s = weight2.T.reshape(KT2, 128, BASE).transpose(1, 0, 2).reshape(128, KT2 * BASE)
    w2s = np.ascontiguousarray(w2s, dtype=f32)
    b1t = np.ascontiguousarray(bias1.reshape(NT1, 128).T, dtype=f32)
    b2t = np.ascontiguousarray(bias2.reshape(NT2, 128).T, dtype=f32)

    # stacked one-hot rhs: rows 0..63 delta(p, px), rows 64..71 delta(p, py%8)
    ohxy = np.zeros((72, CH), dtype=f32)
    ohxy[0:64] = np.tile(np.eye(64, dtype=f32), (1, CH // 64))
    ohxy[64:72] = np.repeat(np.eye(8, dtype=f32), 64, axis=1)
    selm = np.tile(np.eye(4, dtype=f32), (1, 32))           # delta(s, k%4)

    in_maps = []
    for core in range(NCORES):
        bs = [core * BPC + i for i in range(BPC)]
        # block-diagonal cos lhsT per (batch, chunk): cols 0..63 are the 64
        # cos_x features (rows = px slots), cols 64..127 the cos_y features
        # (rows 64..71 = the chunk's 8 py slots)
        cxcy = np.zeros((72, BPC * NCH * 128), dtype=f32)
        for i, b in enumerate(bs):
            for cc in range(NCH):
                c0 = (i * NCH + cc) * 128
                cxcy[0:64, c0:c0 + 64] = cxa[b]
                cxcy[64:72, c0 + 64:c0 + 128] = cya[b, 8 * cc:8 * cc + 8, :]
        gest2 = np.empty((128, 2 * BPC), dtype=f32)
        for kt in range(2):
            for i, b in enumerate(bs):
                gest2[:, kt * BPC + i] = gestalt[b, kt * 128:(kt + 1) * 128]
        in_maps.append({
            "mdf": np.ascontiguousarray(mdf[bs]),
            "cxcy": cxcy, "ohxy": ohxy, "selm": selm,
            "w1a": w1a, "w1bd": w1bd, "w1g": w1g, "gest2": gest2,
            "w2s": w2s, "b1t": b1t, "b2t": b2t,
        })
    return in_maps


last_results = None  # BassKernelResults of the most recent run (for test.py)


def kernel(position, gestalt, mask, depth, weight1, bias1, weight2, bias2,
           trace=False):
    global last_results
    from concourse.bass_utils import run_bass_kernel_spmd

    if "nc" not in _cache:
        _cache["nc"] = _build_nc()
    nc = _cache["nc"]

    in_maps = _host_prep(position, gestalt, mask, depth,
                         np.asarray(weight1), np.asarray(bias1),
                         np.asarray(weight2), np.asarray(bias2))
    res = run_bass_kernel_spmd(nc, in_maps, list(range(NCORES)), trace=trace)
    last_results = res

    out = np.empty((B, BASE, PH, PW), dtype=np.float32)
    for core in range(NCORES):
        o = res.results[core]["out"].reshape(BPC, BASE, PH, PW)
        out[core * BPC:(core + 1) * BPC] = o
    return out


# revision 28
# speedup vs baseline: 1.0357x; 1.0357x over previous
"""Trainium2 Bass kernel for nn_MemoryEfficientRGBDecoderStem.

Reference computation (per batch b of 16):
  - Fourier positional embedding: 32 channels of cos((grid - pos)*std*pi/2*2^f),
    separable in x / y.
  - inp = concat([gestalt (256ch) * mask, embedding (32ch) * mask, depth * mask])
  - space-to-depth 2x2 patchify -> rows (64*64 patches, 289*4=1156 cols)
  - h = silu(p @ W1.T + b1)  (hidden 1024)
  - o = h @ W2.T + b2        (base 256) -> (B, 256, 64, 64)

Strategy: data-parallel over batch, 2 batches per NeuronCore (8 cores).
The GEMM1 contraction is algebraically reduced from K=1156 to K=136:
  * the 1024 gestalt*mask columns collapse to 4 "mask" features with
    per-batch folded weights wg[s,n] = sum_c gestalt[c] * W1[n,(c,s)]
    (wg computed on-device with a small matmul),
  * the 128 cos-embedding columns are built on device via one-hot
    broadcast matmuls (cos tables x one-hot over px/py) times mask,
  * 4 depth*mask features.
All matmuls run in float32r (fast fp32 mode, 1 cycle/row at N>=512).
"""

import numpy as np

# ---- static problem shapes ----
B, G, H, W = 16, 256, 128, 128
NF, S = 16, 2
HID, BASE = 1024, 256
NCORES = 8
BPC = B // NCORES          # batches per core = 2
PH, PW = H // S, W // S    # 64, 64
R = PH * PW                # 4096 patch rows per batch
CH = 512                   # r-chunk size
NCH = R // CH              # 8 chunks per batch
NT1 = HID // 128           # 8 n-tiles for GEMM1
NT2 = BASE // 128          # 2 m-tiles for GEMM2
KT2 = HID // 128           # 8 k-tiles for GEMM2

_cache = {}


def _build_nc():
    import concourse.tile as tile
    from concourse import bacc, mybir

    dt = mybir.dt
    f32, f32r, bf16 = dt.float32, dt.float32r, dt.bfloat16

    nc = bacc.Bacc("TRN2", target_bir_lowering=False, debug=False,
                   num_devices=NCORES)

    # ---- DRAM I/O (per core) ----
    d_mdf = nc.dram_tensor("mdf", [BPC, 8, R], f32, kind="ExternalInput").ap()
    d_cxcy = nc.dram_tensor("cxcy", [72, BPC * NCH * 128], f32,
                            kind="ExternalInput").ap()
    d_ohxy = nc.dram_tensor("ohxy", [72, CH], f32, kind="ExternalInput").ap()
    d_selm = nc.dram_tensor("selm", [4, 128], f32, kind="ExternalInput").ap()
    d_w1a = nc.dram_tensor("w1a", [128, HID], f32, kind="ExternalInput").ap()
    d_w1bd = nc.dram_tensor("w1bd", [4, HID], f32, kind="ExternalInput").ap()
    d_w1g = nc.dram_tensor("w1g", [128, 2 * 4 * HID], f32, kind="ExternalInput").ap()
    d_gest = nc.dram_tensor("gest2", [128, 2 * BPC], f32, kind="ExternalInput").ap()
    d_w2s = nc.dram_tensor("w2s", [128, KT2 * BASE], f32, kind="ExternalInput").ap()
    d_b1t = nc.dram_tensor("b1t", [128, NT1], f32, kind="ExternalInput").ap()
    d_b2t = nc.dram_tensor("b2t", [128, NT2], f32, kind="ExternalInput").ap()
    d_out = nc.dram_tensor("out", [BPC, BASE, R], f32, kind="ExternalOutput").ap()

    with tile.TileContext(nc) as tc:
        with tc.tile_pool(name="const", bufs=1) as cp, \
             tc.tile_pool(name="stage", bufs=2) as stp, \
             tc.tile_pool(name="feat", bufs=3) as fp, \
             tc.tile_pool(name="h", bufs=3) as hp, \
             tc.tile_pool(name="outp", bufs=4) as op, \
             tc.tile_pool(name="ps_feat", bufs=1, space="PSUM") as psf, \
             tc.tile_pool(name="ps_cv", bufs=2, space="PSUM") as pcv, \
             tc.tile_pool(name="ps_g1", bufs=3, space="PSUM") as ps1, \
             tc.tile_pool(name="ps_g2", bufs=2, space="PSUM") as ps2:

            # ---- load + round constants ----
            def load_round(dram_ap, shape, pool=cp, rdt=f32r):
                st = stp.tile(shape, f32, tag="stage")
                nc.sync.dma_start(st[:], dram_ap)
                rt = pool.tile(shape, rdt, tag=f"r{dram_ap.tensor.name}")
                nc.vector.tensor_copy(rt[:], st[:])
                return rt

            w1a_r = load_round(d_w1a, [128, HID])
            w2s_r = load_round(d_w2s, [128, KT2 * BASE], rdt=bf16)
            cxcy_r = load_round(d_cxcy, [72, BPC * NCH * 128])
            ohxy_r = load_round(d_ohxy, [72, CH])
            selm_r = load_round(d_selm, [4, 128])

            b1t = cp.tile([128, NT1], f32)
            nc.sync.dma_start(b1t[:], d_b1t)
            b2t = cp.tile([128, NT2], f32)
            nc.sync.dma_start(b2t[:], d_b2t)

            blhsT = []
            # transient pools for the gestalt fold; close before the main loop
            # (PSUM comes from the ps_g2 pool, which is idle until the loop)
            with tc.tile_pool(name="wg1", bufs=1) as wg1, \
                 tc.tile_pool(name="wg2", bufs=2) as wg2:
                pswg = ps2
                gest_r = load_round(d_gest, [128, 2 * BPC], pool=wg1)
                # wg[b, s*HID+n] = sum_c gestalt[b,c] W1[n,(c,s)],
                # streaming w1g in 512-wide column chunks
                wgst = wg1.tile([BPC, 4 * HID], f32, tag="wgst")
                for sub in range(4 * HID // 512):
                    ps = pswg.tile([BPC, 512], f32, tag="g2")
                    for kt in range(2):
                        wc = wg2.tile([128, 512], f32, tag="wgchunk")
                        nc.sync.dma_start(
                            wc[:],
                            d_w1g[:, kt * 4 * HID + sub * 512:
                                  kt * 4 * HID + (sub + 1) * 512])
                        wcr = wg2.tile([128, 512], f32r, tag="wgchunkr")
                        nc.vector.tensor_copy(wcr[:], wc[:])
                        nc.tensor.matmul(
                            ps[:], gest_r[:, kt * BPC:(kt + 1) * BPC], wcr[:],
                            start=(kt == 0), stop=(kt == 1))
                    nc.scalar.copy(wgst[:, sub * 512:(sub + 1) * 512], ps[:])

                # blhsT per batch: rows 0..3 = wg, rows 4..7 = depth W1.
                # Re-partition (1,4096) -> (4,1024) via a DRAM round-trip.
                with tc.tile_pool(name="wgd", bufs=1, space="DRAM") as wgd:
                    dwg = wgd.tile([BPC, 4, HID], f32, tag="dwg")
                    nc.sync.dma_start(dwg[:], wgst[:].rearrange(
                        "b (s n) -> b s n", s=4))
                    for b in range(BPC):
                        bl = wg1.tile([8, HID], f32, tag=f"blhsT{b}")
                        nc.sync.dma_start(bl[0:4, :], dwg[b])
                        nc.sync.dma_start(bl[4:8, :], d_w1bd)
                        blr = cp.tile([8, HID], f32r, tag=f"blhsTr{b}")
                        nc.vector.tensor_copy(blr[:], bl[:])
                        blhsT.append(blr)

            # ---- per-batch mask/depth features ----
            # mdf rows 0..3 = mask_s[r], rows 4..7 = depth_s[r] -> *= mask.
            # Compute engines can only address partition bases 0/32/64/96,
            # so build the depth*mask product at base 0 and DMA it into
            # partitions 4..7 of the rounded tile.
            mdf_r = []
            with tc.tile_pool(name="mdfstp", bufs=1) as mstp:
                for b in range(BPC):
                    mkf = mstp.tile([4, R], f32, tag=f"maskst{b}")
                    nc.sync.dma_start(mkf[:], d_mdf[b, 0:4])
                    dpf = mstp.tile([4, R], f32, tag="depthst")
                    nc.sync.dma_start(dpf[:], d_mdf[b, 4:8])
                    dpr = mstp.tile([4, R], f32r, tag="depthr")
                    nc.vector.tensor_tensor(dpr[:], dpf[:], mkf[:],
                                            mybir.AluOpType.mult)
                    mr = cp.tile([8, R], f32r, tag=f"mdfr{b}")
                    nc.vector.tensor_copy(mr[0:4, :], mkf[:])
                    nc.sync.dma_start(mr[4:8, :], dpr[:])
                    mdf_r.append(mr)

            # ---- main loop over (batch, r-chunk) ----
            for b in range(BPC):
                for cc in range(NCH):
                    r0 = cc * CH
                    sl = slice(r0, r0 + CH)

                    # mask broadcast to 128 partitions (k = f*4+s -> s = k%4)
                    mrep_ps = psf.tile([128, CH], f32, tag="mrep")
                    nc.tensor.matmul(mrep_ps[:], selm_r[:],
                                     mdf_r[b][0:4, sl], start=True, stop=True)
                    mrep = fp.tile([128, CH], f32, tag="mrep_sb")
                    nc.vector.tensor_copy(mrep[:], mrep_ps[:])

                    # cos values broadcast over the chunk: single K=72 matmul
                    # with block-diagonal lhsT [cx (64 px rows) | cy (8 py
                    # rows)] against the stacked one-hot rhs (identical for
                    # every 512-r chunk).
                    blk = b * NCH + cc
                    cv_ps = pcv.tile([128, CH], f32, tag="cv")
                    nc.tensor.matmul(cv_ps[:],
                                     cxcy_r[:, blk * 128:(blk + 1) * 128],
                                     ohxy_r[:], start=True, stop=True)

                    # embedding features = cos * mask (rounded to float32r)
                    fa = fp.tile([128, CH], f32r, tag="fa")
                    nc.vector.tensor_mul(fa[:], cv_ps[:], mrep[:])

                    # GEMM1 + bias1 + SiLU -> hT chunk (8 n-tiles side by side)
                    hT = hp.tile([128, NT1 * CH], bf16, tag="hT")
                    for nt in range(NT1):
                        g1 = ps1.tile([128, CH], f32, tag="g1")
                        nc.tensor.matmul(g1[:],
                                         w1a_r[:, nt * 128:(nt + 1) * 128],
                                         fa[:], start=True, stop=False)
                        nc.tensor.matmul(g1[:],
                                         blhsT[b][:, nt * 128:(nt + 1) * 128],
                                         mdf_r[b][:, sl], start=False, stop=True)
                        nc.scalar.activation(
                            hT[:, nt * CH:(nt + 1) * CH], g1[:],
                            mybir.ActivationFunctionType.Silu,
                            bias=b1t[:, nt:nt + 1], scale=1.0)

                    # GEMM2 + bias2 -> out chunk
                    for mt in range(NT2):
                        g2 = ps2.tile([128, CH], f32, tag="g2")
                        for kt in range(KT2):
                            nc.tensor.matmul(
                                g2[:],
                                w2s_r[:, kt * BASE + mt * 128:
                                      kt * BASE + (mt + 1) * 128],
                                hT[:, kt * CH:(kt + 1) * CH],
                                start=(kt == 0), stop=(kt == KT2 - 1))
                        ob = op.tile([128, CH], f32, tag="ob")
                        nc.vector.tensor_scalar_add(ob[:], g2[:],
                                                    b2t[:, mt:mt + 1])
                        nc.sync.dma_start(
                            d_out[b, mt * 128:(mt + 1) * 128, sl], ob[:])

    nc.compile()
    return nc


def _host_prep(position, gestalt, mask, depth, weight1, bias1, weight2, bias2):
    """Pure-layout host prep + tiny cos tables. Returns per-core input maps."""
    f32 = np.float32
    # --- cos tables, replicating the reference's fp32 op order exactly ---
    gx = np.arange(W, dtype=f32)
    gx = gx / f32(W - 1)
    gx = gx * f32(2)
    gx = gx - f32(1)          # (128,) == gy since H == W
    x = np.clip(position[:, 0], f32(-1.0), f32(1.0)).astype(f32)[:, None]
    y = np.clip(position[:, 1], f32(-1.0), f32(1.0)).astype(f32)[:, None]
    min_std = f32(1.0 / min(H, W))
    std = (f32(0.1) / np.clip(position[:, 3], min_std, f32(0.5)).astype(f32))[:, None]
    half_pi = f32(np.pi / 2)
    ngx = ((gx[None, :] - x) * std) * half_pi          # (B, 128) fp32
    ngy = ((gx[None, :] - y) * std) * half_pi
    freqs = (f32(2.0) ** np.arange(NF, dtype=f32))[None, :, None]
    ax = (ngx[:, None, :] * freqs).astype(f32)         # (B, NF, 128)
    ay = (ngy[:, None, :] * freqs).astype(f32)
    cosx = np.cos(ax.astype(np.float64)).astype(f32)   # true cos of fp32 angle
    cosy = np.cos(ay.astype(np.float64)).astype(f32)

    # cxa[b, px, f*4+s] = cosx[b, f, 2px + s%2]; cya uses s//2 with py
    cxa = np.empty((B, 64, 64), dtype=f32)
    cya = np.empty((B, 64, 64), dtype=f32)
    px2 = 2 * np.arange(64)
    for s in range(4):
        sx, sy = s % 2, s // 2
        cxa[:, :, np.arange(NF) * 4 + s] = cosx[:, :, px2 + sx].transpose(0, 2, 1)
        cya[:, :, np.arange(NF) * 4 + s] = cosy[:, :, px2 + sy].transpose(0, 2, 1)

    # --- mask/depth patchify: mdf[b, 0:4, r]=mask_s, [4:8]=depth_s (raw) ---
    def patch(a):  # (B,1,H,W) -> (B, 4, R) with s=(sy*2+sx), r=py*64+px
        a6 = a[:, 0].reshape(B, PH, S, PW, S)
        return a6.transpose(0, 2, 4, 1, 3).reshape(B, 4, R).astype(f32)

    mdf = np.concatenate([patch(mask), patch(depth)], axis=1)  # (B, 8, R)

    # --- weight reshuffles ---
    w1e = weight1[:, G * 4:(G + 2 * NF) * 4].reshape(HID, 2 * NF, 4)
    w1a = np.concatenate([
        w1e[:, 0::2, :].transpose(1, 2, 0).reshape(64, HID),   # even ch (cos_x)
        w1e[:, 1::2, :].transpose(1, 2, 0).reshape(64, HID),   # odd ch (cos_y)
    ], axis=0).astype(f32)                                     # (128, HID)
    w1bd = weight1[:, (G + 2 * NF) * 4:].T.astype(f32).copy()  # (4, HID)
    w1g = weight1[:, :G * 4].reshape(HID, G, 4).transpose(1, 2, 0)  # (256,4,HID)
    w1g = w1g.reshape(2, 128, 4 * HID).transpose(1, 0, 2).reshape(128, 2 * 4 * HID)
    w1g = np.ascontiguousarray(w1g, dtype=f32)
    w2s = weight2.T.reshape(KT2, 128, BASE).transpose(1, 0, 2).reshape(128, KT2 * BASE)
    w2s = np.ascontiguousarray(w2s, dtype=f32)
    b1t = np.ascontiguousarray(bias1.reshape(NT1, 128).T, dtype=f32)
    b2t = np.ascontiguousarray(bias2.reshape(NT2, 128).T, dtype=f32)

    # stacked one-hot rhs: rows 0..63 delta(p, px), rows 64..71 delta(p, py%8)
    ohxy = np.zeros((72, CH), dtype=f32)
    ohxy[0:64] = np.tile(np.eye(64, dtype=f32), (1, CH // 64))
    ohxy[64:72] = np.repeat(np.eye(8, dtype=f32), 64, axis=1)
    selm = np.tile(np.eye(4, dtype=f32), (1, 32))           # delta(s, k%4)

    in_maps = []
    for core in range(NCORES):
        bs = [core * BPC + i for i in range(BPC)]
        # block-diagonal cos lhsT per (batch, chunk): cols 0..63 are the 64
        # cos_x features (rows = px slots), cols 64..127 the cos_y features
        # (rows 64..71 = the chunk's 8 py slots)
        cxcy = np.zeros((72, BPC * NCH * 128), dtype=f32)
        for i, b in enumerate(bs):
            for cc in range(NCH):
                c0 = (i * NCH + cc) * 128
                cxcy[0:64, c0:c0 + 64] = cxa[b]
                cxcy[64:72, c0 + 64:c0 + 128] = cya[b, 8 * cc:8 * cc + 8, :]
        gest2 = np.empty((128, 2 * BPC), dtype=f32)
        for kt in range(2):
            for i, b in enumerate(bs):
                gest2[:, kt * BPC + i] = gestalt[b, kt * 128:(kt + 1) * 128]
        in_maps.append({
            "mdf": np.ascontiguousarray(mdf[bs]),
            "cxcy": cxcy, "ohxy": ohxy, "selm": selm,
            "w1a": w1a, "w1bd": w1bd, "w1g": w1g, "gest2": gest2,
            "w2s": w2s, "b1t": b1t, "b2t": b2t,
        })
    return in_maps


last_results = None  # BassKernelResults of the most recent run (for test.py)


def kernel(position, gestalt, mask, depth, weight1, bias1, weight2, bias2,
           trace=False):
    global last_results
    from concourse.bass_utils import run_bass_kernel_spmd

    if "nc" not in _cache:
        _cache["nc"] = _build_nc()
    nc = _cache["nc"]

    in_maps = _host_prep(position, gestalt, mask, depth,
                         np.asarray(weight1), np.asarray(bias1),
                         np.asarray(weight2), np.asarray(bias2))
    res = run_bass_kernel_spmd(nc, in_maps, list(range(NCORES)), trace=trace)
    last_results = res

    out = np.empty((B, BASE, PH, PW), dtype=np.float32)
    for core in range(NCORES):
        o = res.results[core]["out"].reshape(BPC, BASE, PH, PW)
        out[core * BPC:(core + 1) * BPC] = o
    return out


# revision 32
# speedup vs baseline: 1.1652x; 1.1250x over previous
"""Trainium2 Bass kernel for nn_MemoryEfficientRGBDecoderStem.

Reference computation (per batch b of 16):
  - Fourier positional embedding: 32 channels of cos((grid - pos)*std*pi/2*2^f),
    separable in x / y.
  - inp = concat([gestalt (256ch) * mask, embedding (32ch) * mask, depth * mask])
  - space-to-depth 2x2 patchify -> rows (64*64 patches, 289*4=1156 cols)
  - h = silu(p @ W1.T + b1)  (hidden 1024)
  - o = h @ W2.T + b2        (base 256) -> (B, 256, 64, 64)

Strategy: data-parallel over batch, 2 batches per NeuronCore (8 cores).
The GEMM1 contraction is algebraically reduced from K=1156 to K=136:
  * the 1024 gestalt*mask columns collapse to 4 "mask" features with
    per-batch folded weights wg[s,n] = sum_c gestalt[c] * W1[n,(c,s)]
    (wg computed on-device with a small matmul),
  * the 128 cos-embedding columns are built on device via one-hot
    broadcast matmuls (cos tables x one-hot over px/py) times mask,
  * 4 depth*mask features.
All matmuls run in float32r (fast fp32 mode, 1 cycle/row at N>=512).
"""

import numpy as np

# ---- static problem shapes ----
B, G, H, W = 16, 256, 128, 128
NF, S = 16, 2
HID, BASE = 1024, 256
NCORES = 8
BPC = B // NCORES          # batches per core = 2
PH, PW = H // S, W // S    # 64, 64
R = PH * PW                # 4096 patch rows per batch
CH = 512                   # r-chunk size
NCH = R // CH              # 8 chunks per batch
NT1 = HID // 128           # 8 n-tiles for GEMM1
NT2 = BASE // 128          # 2 m-tiles for GEMM2
KT2 = HID // 128           # 8 k-tiles for GEMM2

_cache = {}


def _build_nc():
    import concourse.tile as tile
    from concourse import bacc, mybir

    dt = mybir.dt
    f32, f32r, bf16 = dt.float32, dt.float32r, dt.bfloat16

    nc = bacc.Bacc("TRN2", target_bir_lowering=False, debug=False,
                   num_devices=NCORES)

    # ---- DRAM I/O (per core) ----
    d_mdf = nc.dram_tensor("mdf", [BPC, 8, R], f32, kind="ExternalInput").ap()
    d_cxcy = nc.dram_tensor("cxcy", [72, BPC * NCH * 128], f32,
                            kind="ExternalInput").ap()
    d_ohxy = nc.dram_tensor("ohxy", [72, CH], f32, kind="ExternalInput").ap()
    d_selm = nc.dram_tensor("selm", [4, 128], f32, kind="ExternalInput").ap()
    d_w1a = nc.dram_tensor("w1a", [128, HID], f32, kind="ExternalInput").ap()
    d_w1bd = nc.dram_tensor("w1bd", [4, HID], f32, kind="ExternalInput").ap()
    d_w1g = nc.dram_tensor("w1g", [128, 2 * 4 * HID], f32, kind="ExternalInput").ap()
    d_gest = nc.dram_tensor("gest2", [128, 2 * BPC], f32, kind="ExternalInput").ap()
    d_w2s = nc.dram_tensor("w2s", [128, KT2 * BASE], f32, kind="ExternalInput").ap()
    d_b1t = nc.dram_tensor("b1t", [128, NT1], f32, kind="ExternalInput").ap()
    d_b2t = nc.dram_tensor("b2t", [128, NT2], f32, kind="ExternalInput").ap()
    d_out = nc.dram_tensor("out", [BPC, BASE, R], f32, kind="ExternalOutput").ap()

    with tile.TileContext(nc) as tc:
        with tc.tile_pool(name="const", bufs=1) as cp, \
             tc.tile_pool(name="stage", bufs=2) as stp, \
             tc.tile_pool(name="feat", bufs=3) as fp, \
             tc.tile_pool(name="h", bufs=3) as hp, \
             tc.tile_pool(name="outp", bufs=4) as op, \
             tc.tile_pool(name="ps_feat", bufs=1, space="PSUM") as psf, \
             tc.tile_pool(name="ps_cv", bufs=2, space="PSUM") as pcv, \
             tc.tile_pool(name="ps_g1", bufs=3, space="PSUM") as ps1, \
             tc.tile_pool(name="ps_g2", bufs=2, space="PSUM") as ps2:

            # ---- load + round constants ----
            def load_round(dram_ap, shape, pool=cp, rdt=f32r):
                st = stp.tile(shape, f32, tag="stage")
                nc.sync.dma_start(st[:], dram_ap)
                rt = pool.tile(shape, rdt, tag=f"r{dram_ap.tensor.name}")
                nc.vector.tensor_copy(rt[:], st[:])
                return rt

            w1a_r = load_round(d_w1a, [128, HID])
            w2s_r = load_round(d_w2s, [128, KT2 * BASE], rdt=bf16)
            cxcy_r = load_round(d_cxcy, [72, BPC * NCH * 128])
            ohxy_r = load_round(d_ohxy, [72, CH])
            selm_r = load_round(d_selm, [4, 128])

            b1t = cp.tile([128, NT1], f32)
            nc.sync.dma_start(b1t[:], d_b1t)
            b2t = cp.tile([128, NT2], f32)
            nc.sync.dma_start(b2t[:], d_b2t)

            blhsT = []
            # transient pools for the gestalt fold; close before the main loop
            # (PSUM comes from the ps_g2 pool, which is idle until the loop)
            with tc.tile_pool(name="wg1", bufs=1) as wg1, \
                 tc.tile_pool(name="wg2", bufs=2) as wg2:
                pswg = ps2
                gest_r = load_round(d_gest, [128, 2 * BPC], pool=wg1)
                # wg[b, s*HID+n] = sum_c gestalt[b,c] W1[n,(c,s)],
                # streaming w1g in 512-wide column chunks
                wgst = wg1.tile([BPC, 4 * HID], f32, tag="wgst")
                for sub in range(4 * HID // 512):
                    ps = pswg.tile([BPC, 512], f32, tag="g2")
                    for kt in range(2):
                        wc = wg2.tile([128, 512], f32, tag="wgchunk")
                        nc.sync.dma_start(
                            wc[:],
                            d_w1g[:, kt * 4 * HID + sub * 512:
                                  kt * 4 * HID + (sub + 1) * 512])
                        wcr = wg2.tile([128, 512], f32r, tag="wgchunkr")
                        nc.vector.tensor_copy(wcr[:], wc[:])
                        nc.tensor.matmul(
                            ps[:], gest_r[:, kt * BPC:(kt + 1) * BPC], wcr[:],
                            start=(kt == 0), stop=(kt == 1))
                    nc.scalar.copy(wgst[:, sub * 512:(sub + 1) * 512], ps[:])

                # blhsT per batch: rows 0..3 = wg, rows 4..7 = depth W1.
                # Re-partition (1,4096) -> (4,1024) via a DRAM round-trip.
                with tc.tile_pool(name="wgd", bufs=1, space="DRAM") as wgd:
                    dwg = wgd.tile([BPC, 4, HID], f32, tag="dwg")
                    nc.sync.dma_start(dwg[:], wgst[:].rearrange(
                        "b (s n) -> b s n", s=4))
                    for b in range(BPC):
                        bl = wg1.tile([8, HID], f32, tag=f"blhsT{b}")
                        nc.sync.dma_start(bl[0:4, :], dwg[b])
                        nc.sync.dma_start(bl[4:8, :], d_w1bd)
                        blr = cp.tile([8, HID], f32r, tag=f"blhsTr{b}")
                        nc.vector.tensor_copy(blr[:], bl[:])
                        blhsT.append(blr)

            # ---- per-batch mask/depth features ----
            # mdf rows 0..3 = mask_s[r], rows 4..7 = depth_s[r] -> *= mask.
            # Compute engines can only address partition bases 0/32/64/96,
            # so build the depth*mask product at base 0 and DMA it into
            # partitions 4..7 of the rounded tile.
            mdf_r = []
            with tc.tile_pool(name="mdfstp", bufs=1) as mstp:
                for b in range(BPC):
                    mkf = mstp.tile([4, R], f32, tag=f"maskst{b}")
                    nc.sync.dma_start(mkf[:], d_mdf[b, 0:4])
                    dpf = mstp.tile([4, R], f32, tag="depthst")
                    nc.sync.dma_start(dpf[:], d_mdf[b, 4:8])
                    dpr = mstp.tile([4, R], f32r, tag="depthr")
                    nc.vector.tensor_tensor(dpr[:], dpf[:], mkf[:],
                                            mybir.AluOpType.mult)
                    mr = cp.tile([8, R], f32r, tag=f"mdfr{b}")
                    nc.vector.tensor_copy(mr[0:4, :], mkf[:])
                    nc.sync.dma_start(mr[4:8, :], dpr[:])
                    mdf_r.append(mr)

            # ---- main loop over (batch, r-chunk) ----
            for b in range(BPC):
                for cc in range(NCH):
                    r0 = cc * CH
                    sl = slice(r0, r0 + CH)

                    # mask broadcast to 128 partitions (k = f*4+s -> s = k%4)
                    mrep_ps = psf.tile([128, CH], f32, tag="mrep")
                    nc.tensor.matmul(mrep_ps[:], selm_r[:],
                                     mdf_r[b][0:4, sl], start=True, stop=True)
                    mrep = fp.tile([128, CH], f32, tag="mrep_sb")
                    nc.vector.tensor_copy(mrep[:], mrep_ps[:])

                    # cos values broadcast over the chunk: single K=72 matmul
                    # with block-diagonal lhsT [cx (64 px rows) | cy (8 py
                    # rows)] against the stacked one-hot rhs (identical for
                    # every 512-r chunk).
                    blk = b * NCH + cc
                    cv_ps = pcv.tile([128, CH], f32, tag="cv")
                    nc.tensor.matmul(cv_ps[:],
                                     cxcy_r[:, blk * 128:(blk + 1) * 128],
                                     ohxy_r[:], start=True, stop=True)

                    # embedding features = cos * mask (rounded to float32r)
                    fa = fp.tile([128, CH], f32r, tag="fa")
                    nc.vector.tensor_mul(fa[:], cv_ps[:], mrep[:])

                    # GEMM1 + bias1 + SiLU -> hT chunk (8 n-tiles side by
                    # side). Matmuls are emitted pair-interleaved
                    # (A0,A1,B0,B1,...) so consecutive matmuls never
                    # accumulate into the same PSUM bank — the drain of one
                    # bank hides under the fill of the other.
                    hT = hp.tile([128, NT1 * CH], bf16, tag="hT")
                    for np_ in range(NT1 // 2):
                        nts = (2 * np_, 2 * np_ + 1)
                        g1s = []
                        for nt in nts:
                            g1 = ps1.tile([128, CH], f32, tag="g1")
                            nc.tensor.matmul(g1[:],
                                             w1a_r[:, nt * 128:(nt + 1) * 128],
                                             fa[:], start=True, stop=False)
                            g1s.append(g1)
                        for nt, g1 in zip(nts, g1s):
                            nc.tensor.matmul(g1[:],
                                             blhsT[b][:, nt * 128:(nt + 1) * 128],
                                             mdf_r[b][:, sl],
                                             start=False, stop=True)
                            nc.scalar.activation(
                                hT[:, nt * CH:(nt + 1) * CH], g1[:],
                                mybir.ActivationFunctionType.Silu,
                                bias=b1t[:, nt:nt + 1], scale=1.0)

                    # GEMM2 + bias2 -> out chunk; kt outer / mt inner so the
                    # two accumulating banks alternate every matmul.
                    g2s = [ps2.tile([128, CH], f32, tag="g2",
                                    name=f"g2_{b}_{cc}_{mt}")
                           for mt in range(NT2)]
                    for kt in range(KT2):
                        for mt in range(NT2):
                            nc.tensor.matmul(
                                g2s[mt][:],
                                w2s_r[:, kt * BASE + mt * 128:
                                      kt * BASE + (mt + 1) * 128],
                                hT[:, kt * CH:(kt + 1) * CH],
                                start=(kt == 0), stop=(kt == KT2 - 1))
                    for mt in range(NT2):
                        ob = op.tile([128, CH], f32, tag="ob")
                        nc.vector.tensor_scalar_add(ob[:], g2s[mt][:],
                                                    b2t[:, mt:mt + 1])
                        nc.sync.dma_start(
                            d_out[b, mt * 128:(mt + 1) * 128, sl], ob[:])

    nc.compile()
    return nc


def _host_prep(position, gestalt, mask, depth, weight1, bias1, weight2, bias2):
    """Pure-layout host prep + tiny cos tables. Returns per-core input maps."""
    f32 = np.float32
    # --- cos tables, replicating the reference's fp32 op order exactly ---
    gx = np.arange(W, dtype=f32)
    gx = gx / f32(W - 1)
    gx = gx * f32(2)
    gx = gx - f32(1)          # (128,) == gy since H == W
    x = np.clip(position[:, 0], f32(-1.0), f32(1.0)).astype(f32)[:, None]
    y = np.clip(position[:, 1], f32(-1.0), f32(1.0)).astype(f32)[:, None]
    min_std = f32(1.0 / min(H, W))
    std = (f32(0.1) / np.clip(position[:, 3], min_std, f32(0.5)).astype(f32))[:, None]
    half_pi = f32(np.pi / 2)
    ngx = ((gx[None, :] - x) * std) * half_pi          # (B, 128) fp32
    ngy = ((gx[None, :] - y) * std) * half_pi
    freqs = (f32(2.0) ** np.arange(NF, dtype=f32))[None, :, None]
    ax = (ngx[:, None, :] * freqs).astype(f32)         # (B, NF, 128)
    ay = (ngy[:, None, :] * freqs).astype(f32)
    cosx = np.cos(ax.astype(np.float64)).astype(f32)   # true cos of fp32 angle
    cosy = np.cos(ay.astype(np.float64)).astype(f32)

    # cxa[b, px, f*4+s] = cosx[b, f, 2px + s%2]; cya uses s//2 with py
    cxa = np.empty((B, 64, 64), dtype=f32)
    cya = np.empty((B, 64, 64), dtype=f32)
    px2 = 2 * np.arange(64)
    for s in range(4):
        sx, sy = s % 2, s // 2
        cxa[:, :, np.arange(NF) * 4 + s] = cosx[:, :, px2 + sx].transpose(0, 2, 1)
        cya[:, :, np.arange(NF) * 4 + s] = cosy[:, :, px2 + sy].transpose(0, 2, 1)

    # --- mask/depth patchify: mdf[b, 0:4, r]=mask_s, [4:8]=depth_s (raw) ---
    def patch(a):  # (B,1,H,W) -> (B, 4, R) with s=(sy*2+sx), r=py*64+px
        a6 = a[:, 0].reshape(B, PH, S, PW, S)
        return a6.transpose(0, 2, 4, 1, 3).reshape(B, 4, R).astype(f32)

    mdf = np.concatenate([patch(mask), patch(depth)], axis=1)  # (B, 8, R)

    # --- weight reshuffles ---
    w1e = weight1[:, G * 4:(G + 2 * NF) * 4].reshape(HID, 2 * NF, 4)
    w1a = np.concatenate([
        w1e[:, 0::2, :].transpose(1, 2, 0).reshape(64, HID),   # even ch (cos_x)
        w1e[:, 1::2, :].transpose(1, 2, 0).reshape(64, HID),   # odd ch (cos_y)
    ], axis=0).astype(f32)                                     # (128, HID)
    w1bd = weight1[:, (G + 2 * NF) * 4:].T.astype(f32).copy()  # (4, HID)
    w1g = weight1[:, :G * 4].reshape(HID, G, 4).transpose(1, 2, 0)  # (256,4,HID)
    w1g = w1g.reshape(2, 128, 4 * HID).transpose(1, 0, 2).reshape(128, 2 * 4 * HID)
    w1g = np.ascontiguousarray(w1g, dtype=f32)
    w2s = weight2.T.reshape(KT2, 128, BASE).transpose(1, 0, 2).reshape(128, KT2 * BASE)
    w2s = np.ascontiguousarray(w2s, dtype=f32)
    b1t = np.ascontiguousarray(bias1.reshape(NT1, 128).T, dtype=f32)
    b2t = np.ascontiguousarray(bias2.reshape(NT2, 128).T, dtype=f32)

    # stacked one-hot rhs: rows 0..63 delta(p, px), rows 64..71 delta(p, py%8)
    ohxy = np.zeros((72, CH), dtype=f32)
    ohxy[0:64] = np.tile(np.eye(64, dtype=f32), (1, CH // 64))
    ohxy[64:72] = np.repeat(np.eye(8, dtype=f32), 64, axis=1)
    selm = np.tile(np.eye(4, dtype=f32), (1, 32))           # delta(s, k%4)

    in_maps = []
    for core in range(NCORES):
        bs = [core * BPC + i for i in range(BPC)]
        # block-diagonal cos lhsT per (batch, chunk): cols 0..63 are the 64
        # cos_x features (rows = px slots), cols 64..127 the cos_y features
        # (rows 64..71 = the chunk's 8 py slots)
        cxcy = np.zeros((72, BPC * NCH * 128), dtype=f32)
        for i, b in enumerate(bs):
            for cc in range(NCH):
                c0 = (i * NCH + cc) * 128
                cxcy[0:64, c0:c0 + 64] = cxa[b]
                cxcy[64:72, c0 + 64:c0 + 128] = cya[b, 8 * cc:8 * cc + 8, :]
        gest2 = np.empty((128, 2 * BPC), dtype=f32)
        for kt in range(2):
            for i, b in enumerate(bs):
                gest2[:, kt * BPC + i] = gestalt[b, kt * 128:(kt + 1) * 128]
        in_maps.append({
            "mdf": np.ascontiguousarray(mdf[bs]),
            "cxcy": cxcy, "ohxy": ohxy, "selm": selm,
            "w1a": w1a, "w1bd": w1bd, "w1g": w1g, "gest2": gest2,
            "w2s": w2s, "b1t": b1t, "b2t": b2t,
        })
    return in_maps


last_results = None  # BassKernelResults of the most recent run (for test.py)


def _enable_ldw_opt():
    """Flip walrus's --enable-ldw-opt to true so LDWEIGHTS can target the
    PE background weight buffer (weight-load / matmul overlap)."""
    import concourse.bass_utils as bu
    if getattr(bu, "_ldw_opt_patched", False):
        return
    orig = bu.run_command

    def patched(cmd, *a, **kw):
        if isinstance(cmd, list):
            cmd = ["--enable-ldw-opt=true" if c == "--enable-ldw-opt=false"
                   else c for c in cmd]
        return orig(cmd, *a, **kw)

    bu.run_command = patched
    bu._ldw_opt_patched = True


def kernel(position, gestalt, mask, depth, weight1, bias1, weight2, bias2,
           trace=False):
    global last_results
    from concourse.bass_utils import run_bass_kernel_spmd

    if "nc" not in _cache:
        _cache["nc"] = _build_nc()
    nc = _cache["nc"]

    in_maps = _host_prep(position, gestalt, mask, depth,
                         np.asarray(weight1), np.asarray(bias1),
                         np.asarray(weight2), np.asarray(bias2))
    res = run_bass_kernel_spmd(nc, in_maps, list(range(NCORES)), trace=trace)
    last_results = res

    out = np.empty((B, BASE, PH, PW), dtype=np.float32)
    for core in range(NCORES):
        o = res.results[core]["out"].reshape(BPC, BASE, PH, PW)
        out[core * BPC:(core + 1) * BPC] = o
    return out


# revision 35
# speedup vs baseline: 1.2442x; 1.0678x over previous
"""Trainium2 Bass kernel for nn_MemoryEfficientRGBDecoderStem.

Reference computation (per batch b of 16):
  - Fourier positional embedding: 32 channels of cos((grid - pos)*std*pi/2*2^f),
    separable in x / y.
  - inp = concat([gestalt (256ch) * mask, embedding (32ch) * mask, depth * mask])
  - space-to-depth 2x2 patchify -> rows (64*64 patches, 289*4=1156 cols)
  - h = silu(p @ W1.T + b1)  (hidden 1024)
  - o = h @ W2.T + b2        (base 256) -> (B, 256, 64, 64)

Strategy: data-parallel over batch, 2 batches per NeuronCore (8 cores).
The GEMM1 contraction is algebraically reduced from K=1156 to K=136:
  * the 1024 gestalt*mask columns collapse to 4 "mask" features with
    per-batch folded weights wg[s,n] = sum_c gestalt[c] * W1[n,(c,s)]
    (wg computed on-device with a small matmul),
  * the 128 cos-embedding columns are built on device via one-hot
    broadcast matmuls (cos tables x one-hot over px/py) times mask,
  * 4 depth*mask features.
All matmuls run in float32r (fast fp32 mode, 1 cycle/row at N>=512).
"""

import numpy as np

# ---- static problem shapes ----
B, G, H, W = 16, 256, 128, 128
NF, S = 16, 2
HID, BASE = 1024, 256
NCORES = 8
BPC = B // NCORES          # batches per core = 2
PH, PW = H // S, W // S    # 64, 64
R = PH * PW                # 4096 patch rows per batch
CH = 512                   # r-chunk size
NCH = R // CH              # 8 chunks per batch
NT1 = HID // 128           # 8 n-tiles for GEMM1
NT2 = BASE // 128          # 2 m-tiles for GEMM2
KT2 = HID // 128           # 8 k-tiles for GEMM2

_cache = {}


def _build_nc():
    import concourse.tile as tile
    from concourse import bacc, mybir

    dt = mybir.dt
    f32, f32r, bf16 = dt.float32, dt.float32r, dt.bfloat16

    nc = bacc.Bacc("TRN2", target_bir_lowering=False, debug=False,
                   num_devices=NCORES)

    # ---- DRAM I/O (per core) ----
    d_mdf = nc.dram_tensor("mdf", [BPC, 8, R], f32, kind="ExternalInput").ap()
    d_cxcy = nc.dram_tensor("cxcy", [72, BPC * NCH * 128], f32,
                            kind="ExternalInput").ap()
    d_ohxy = nc.dram_tensor("ohxy", [72, CH], f32, kind="ExternalInput").ap()
    d_selm = nc.dram_tensor("selm", [4, 128], f32, kind="ExternalInput").ap()
    d_w1a = nc.dram_tensor("w1a", [128, HID], f32, kind="ExternalInput").ap()
    d_w1bd = nc.dram_tensor("w1bd", [4, HID], f32, kind="ExternalInput").ap()
    d_w1g = nc.dram_tensor("w1g", [128, 2 * 4 * HID], f32, kind="ExternalInput").ap()
    d_gest = nc.dram_tensor("gest2", [128, 2 * BPC], f32, kind="ExternalInput").ap()
    d_w2s = nc.dram_tensor("w2s", [128, KT2 * BASE], f32, kind="ExternalInput").ap()
    d_b1t = nc.dram_tensor("b1t", [128, NT1], f32, kind="ExternalInput").ap()
    d_b2t = nc.dram_tensor("b2t", [128, NT2], f32, kind="ExternalInput").ap()
    d_out = nc.dram_tensor("out", [BPC, BASE, R], f32, kind="ExternalOutput").ap()

    with tile.TileContext(nc) as tc:
        with tc.tile_pool(name="const", bufs=1) as cp, \
             tc.tile_pool(name="stage", bufs=2) as stp, \
             tc.tile_pool(name="feat", bufs=3) as fp, \
             tc.tile_pool(name="h", bufs=3) as hp, \
             tc.tile_pool(name="outp", bufs=4) as op, \
             tc.tile_pool(name="ps_feat", bufs=1, space="PSUM") as psf, \
             tc.tile_pool(name="ps_cv", bufs=2, space="PSUM") as pcv, \
             tc.tile_pool(name="ps_g1", bufs=3, space="PSUM") as ps1, \
             tc.tile_pool(name="ps_g2", bufs=2, space="PSUM") as ps2:

            # ---- load + round constants ----
            def load_round(dram_ap, shape, pool=cp, rdt=f32r):
                st = stp.tile(shape, f32, tag="stage")
                nc.sync.dma_start(st[:], dram_ap)
                rt = pool.tile(shape, rdt, tag=f"r{dram_ap.tensor.name}")
                nc.vector.tensor_copy(rt[:], st[:])
                return rt

            w1a_r = load_round(d_w1a, [128, HID])
            w2s_r = load_round(d_w2s, [128, KT2 * BASE], rdt=bf16)
            cxcy_r = load_round(d_cxcy, [72, BPC * NCH * 128])
            ohxy_r = load_round(d_ohxy, [72, CH])
            selm_r = load_round(d_selm, [4, 128])

            b1t = cp.tile([128, NT1], f32)
            nc.sync.dma_start(b1t[:], d_b1t)
            b2t = cp.tile([128, NT2], f32)
            nc.sync.dma_start(b2t[:], d_b2t)

            blhsT = []
            # transient pools for the gestalt fold; close before the main loop
            # (PSUM comes from the ps_g2 pool, which is idle until the loop)
            with tc.tile_pool(name="wg1", bufs=1) as wg1, \
                 tc.tile_pool(name="wg2", bufs=4) as wg2:
                pswg = ps2
                gest_r = load_round(d_gest, [128, 2 * BPC], pool=wg1)
                # wg[b, s*HID+n] = sum_c gestalt[b,c] W1[n,(c,s)],
                # streaming w1g in 512-wide column chunks
                wgst = wg1.tile([BPC, 4 * HID], f32, tag="wgst")
                for sub in range(4 * HID // 512):
                    ps = pswg.tile([BPC, 512], f32, tag="g2")
                    for kt in range(2):
                        wc = wg2.tile([128, 512], f32, tag="wgchunk")
                        nc.sync.dma_start(
                            wc[:],
                            d_w1g[:, kt * 4 * HID + sub * 512:
                                  kt * 4 * HID + (sub + 1) * 512])
                        wcr = wg2.tile([128, 512], f32r, tag="wgchunkr")
                        nc.vector.tensor_copy(wcr[:], wc[:])
                        nc.tensor.matmul(
                            ps[:], gest_r[:, kt * BPC:(kt + 1) * BPC], wcr[:],
                            start=(kt == 0), stop=(kt == 1))
                    nc.scalar.copy(wgst[:, sub * 512:(sub + 1) * 512], ps[:])

                # blhsT per batch: rows 0..3 = wg, rows 4..7 = depth W1.
                # Re-partition (1,4096) -> (4,1024) via a DRAM round-trip.
                with tc.tile_pool(name="wgd", bufs=1, space="DRAM") as wgd:
                    dwg = wgd.tile([BPC, 4, HID], f32, tag="dwg")
                    nc.sync.dma_start(dwg[:], wgst[:].rearrange(
                        "b (s n) -> b s n", s=4))
                    for b in range(BPC):
                        bl = wg1.tile([8, HID], f32, tag=f"blhsT{b}")
                        nc.sync.dma_start(bl[0:4, :], dwg[b])
                        nc.sync.dma_start(bl[4:8, :], d_w1bd)
                        blr = cp.tile([8, HID], f32r, tag=f"blhsTr{b}")
                        nc.vector.tensor_copy(blr[:], bl[:])
                        blhsT.append(blr)

            # ---- per-batch mask/depth features ----
            # mdf rows 0..3 = mask_s[r], rows 4..7 = depth_s[r] -> *= mask.
            # Compute engines can only address partition bases 0/32/64/96,
            # so build the depth*mask product at base 0 and DMA it into
            # partitions 4..7 of the rounded tile.
            mdf_r = []
            with tc.tile_pool(name="mdfstp", bufs=1) as mstp:
                for b in range(BPC):
                    mkf = mstp.tile([4, R], f32, tag=f"maskst{b}")
                    nc.sync.dma_start(mkf[:], d_mdf[b, 0:4])
                    dpf = mstp.tile([4, R], f32, tag="depthst")
                    nc.sync.dma_start(dpf[:], d_mdf[b, 4:8])
                    dpr = mstp.tile([4, R], f32r, tag="depthr")
                    nc.vector.tensor_tensor(dpr[:], dpf[:], mkf[:],
                                            mybir.AluOpType.mult)
                    mr = cp.tile([8, R], f32r, tag=f"mdfr{b}")
                    nc.vector.tensor_copy(mr[0:4, :], mkf[:])
                    nc.sync.dma_start(mr[4:8, :], dpr[:])
                    mdf_r.append(mr)

            # ---- main loop over (batch, r-chunk), software-pipelined:
            # the feature build for chunk c+1 is emitted before chunk c's
            # GEMMs so the PE never waits on the DVE feature product.
            chunks = [(b, cc) for b in range(BPC) for cc in range(NCH)]

            def build_feat(b, cc, idx):
                sl = slice(cc * CH, (cc + 1) * CH)
                # mask broadcast to 128 partitions (k -> s = k%4)
                mrep_ps = psf.tile([128, CH], f32, tag="mrep",
                                   name=f"mrep_ps_{idx}")
                nc.tensor.matmul(mrep_ps[:], selm_r[:],
                                 mdf_r[b][0:4, sl], start=True, stop=True)
                mrep = fp.tile([128, CH], f32, tag="mrep_sb",
                               name=f"mrep_{idx}")
                nc.vector.tensor_copy(mrep[:], mrep_ps[:])
                # cos values broadcast over the chunk: single K=72 matmul
                # with block-diagonal lhsT [cx (64 px rows) | cy (8 py rows)]
                # against the stacked one-hot rhs (chunk-invariant pattern).
                blk = b * NCH + cc
                cv_ps = pcv.tile([128, CH], f32, tag="cv",
                                 name=f"cv_ps_{idx}")
                nc.tensor.matmul(cv_ps[:],
                                 cxcy_r[:, blk * 128:(blk + 1) * 128],
                                 ohxy_r[:], start=True, stop=True)
                # embedding features = cos * mask (rounded to float32r)
                fa = fp.tile([128, CH], f32r, tag="fa", name=f"fa_{idx}")
                nc.vector.tensor_mul(fa[:], cv_ps[:], mrep[:])
                return fa

            fa_next = build_feat(*chunks[0], 0)
            for idx, (b, cc) in enumerate(chunks):
                r0 = cc * CH
                sl = slice(r0, r0 + CH)
                fa = fa_next
                if idx + 1 < len(chunks):
                    fa_next = build_feat(*chunks[idx + 1], idx + 1)

                # GEMM1 + bias1 + SiLU -> hT chunk (8 n-tiles side by
                # side). Matmuls are emitted pair-interleaved
                # (A0,A1,B0,B1,...) so consecutive matmuls never
                # accumulate into the same PSUM bank — the drain of one
                # bank hides under the fill of the other.
                if True:
                    hT = hp.tile([128, NT1 * CH], bf16, tag="hT",
                                 name=f"hT_{idx}")
                    for np_ in range(NT1 // 2):
                        nts = (2 * np_, 2 * np_ + 1)
                        g1s = []
                        for nt in nts:
                            g1 = ps1.tile([128, CH], f32, tag="g1")
                            nc.tensor.matmul(g1[:],
                                             w1a_r[:, nt * 128:(nt + 1) * 128],
                                             fa[:], start=True, stop=False)
                            g1s.append(g1)
                        for nt, g1 in zip(nts, g1s):
                            nc.tensor.matmul(g1[:],
                                             blhsT[b][:, nt * 128:(nt + 1) * 128],
                                             mdf_r[b][:, sl],
                                             start=False, stop=True)
                            nc.scalar.activation(
                                hT[:, nt * CH:(nt + 1) * CH], g1[:],
                                mybir.ActivationFunctionType.Silu,
                                bias=b1t[:, nt:nt + 1], scale=1.0)

                    # GEMM2 + bias2 -> out chunk; kt outer / mt inner so the
                    # two accumulating banks alternate every matmul.
                    g2s = [ps2.tile([128, CH], f32, tag="g2",
                                    name=f"g2_{idx}_{mt}")
                           for mt in range(NT2)]
                    for kt in range(KT2):
                        for mt in range(NT2):
                            nc.tensor.matmul(
                                g2s[mt][:],
                                w2s_r[:, kt * BASE + mt * 128:
                                      kt * BASE + (mt + 1) * 128],
                                hT[:, kt * CH:(kt + 1) * CH],
                                start=(kt == 0), stop=(kt == KT2 - 1))
                    for mt in range(NT2):
                        ob = op.tile([128, CH], f32, tag="ob")
                        nc.vector.tensor_scalar_add(ob[:], g2s[mt][:],
                                                    b2t[:, mt:mt + 1])
                        nc.sync.dma_start(
                            d_out[b, mt * 128:(mt + 1) * 128, sl], ob[:])

    nc.compile()
    return nc


def _host_prep(position, gestalt, mask, depth, weight1, bias1, weight2, bias2):
    """Pure-layout host prep + tiny cos tables. Returns per-core input maps."""
    f32 = np.float32
    # --- cos tables, replicating the reference's fp32 op order exactly ---
    gx = np.arange(W, dtype=f32)
    gx = gx / f32(W - 1)
    gx = gx * f32(2)
    gx = gx - f32(1)          # (128,) == gy since H == W
    x = np.clip(position[:, 0], f32(-1.0), f32(1.0)).astype(f32)[:, None]
    y = np.clip(position[:, 1], f32(-1.0), f32(1.0)).astype(f32)[:, None]
    min_std = f32(1.0 / min(H, W))
    std = (f32(0.1) / np.clip(position[:, 3], min_std, f32(0.5)).astype(f32))[:, None]
    half_pi = f32(np.pi / 2)
    ngx = ((gx[None, :] - x) * std) * half_pi          # (B, 128) fp32
    ngy = ((gx[None, :] - y) * std) * half_pi
    freqs = (f32(2.0) ** np.arange(NF, dtype=f32))[None, :, None]
    ax = (ngx[:, None, :] * freqs).astype(f32)         # (B, NF, 128)
    ay = (ngy[:, None, :] * freqs).astype(f32)
    cosx = np.cos(ax.astype(np.float64)).astype(f32)   # true cos of fp32 angle
    cosy = np.cos(ay.astype(np.float64)).astype(f32)

    # cxa[b, px, f*4+s] = cosx[b, f, 2px + s%2]; cya uses s//2 with py
    cxa = np.empty((B, 64, 64), dtype=f32)
    cya = np.empty((B, 64, 64), dtype=f32)
    px2 = 2 * np.arange(64)
    for s in range(4):
        sx, sy = s % 2, s // 2
        cxa[:, :, np.arange(NF) * 4 + s] = cosx[:, :, px2 + sx].transpose(0, 2, 1)
        cya[:, :, np.arange(NF) * 4 + s] = cosy[:, :, px2 + sy].transpose(0, 2, 1)

    # --- mask/depth patchify: mdf[b, 0:4, r]=mask_s, [4:8]=depth_s (raw) ---
    def patch(a):  # (B,1,H,W) -> (B, 4, R) with s=(sy*2+sx), r=py*64+px
        a6 = a[:, 0].reshape(B, PH, S, PW, S)
        return a6.transpose(0, 2, 4, 1, 3).reshape(B, 4, R).astype(f32)

    mdf = np.concatenate([patch(mask), patch(depth)], axis=1)  # (B, 8, R)

    # --- weight reshuffles ---
    w1e = weight1[:, G * 4:(G + 2 * NF) * 4].reshape(HID, 2 * NF, 4)
    w1a = np.concatenate([
        w1e[:, 0::2, :].transpose(1, 2, 0).reshape(64, HID),   # even ch (cos_x)
        w1e[:, 1::2, :].transpose(1, 2, 0).reshape(64, HID),   # odd ch (cos_y)
    ], axis=0).astype(f32)                                     # (128, HID)
    w1bd = weight1[:, (G + 2 * NF) * 4:].T.astype(f32).copy()  # (4, HID)
    w1g = weight1[:, :G * 4].reshape(HID, G, 4).transpose(1, 2, 0)  # (256,4,HID)
    w1g = w1g.reshape(2, 128, 4 * HID).transpose(1, 0, 2).reshape(128, 2 * 4 * HID)
    w1g = np.ascontiguousarray(w1g, dtype=f32)
    w2s = weight2.T.reshape(KT2, 128, BASE).transpose(1, 0, 2).reshape(128, KT2 * BASE)
    w2s = np.ascontiguousarray(w2s, dtype=f32)
    b1t = np.ascontiguousarray(bias1.reshape(NT1, 128).T, dtype=f32)
    b2t = np.ascontiguousarray(bias2.reshape(NT2, 128).T, dtype=f32)

    # stacked one-hot rhs: rows 0..63 delta(p, px), rows 64..71 delta(p, py%8)
    ohxy = np.zeros((72, CH), dtype=f32)
    ohxy[0:64] = np.tile(np.eye(64, dtype=f32), (1, CH // 64))
    ohxy[64:72] = np.repeat(np.eye(8, dtype=f32), 64, axis=1)
    selm = np.tile(np.eye(4, dtype=f32), (1, 32))           # delta(s, k%4)

    in_maps = []
    for core in range(NCORES):
        bs = [core * BPC + i for i in range(BPC)]
        # block-diagonal cos lhsT per (batch, chunk): cols 0..63 are the 64
        # cos_x features (rows = px slots), cols 64..127 the cos_y features
        # (rows 64..71 = the chunk's 8 py slots)
        cxcy = np.zeros((72, BPC * NCH * 128), dtype=f32)
        for i, b in enumerate(bs):
            for cc in range(NCH):
                c0 = (i * NCH + cc) * 128
                cxcy[0:64, c0:c0 + 64] = cxa[b]
                cxcy[64:72, c0 + 64:c0 + 128] = cya[b, 8 * cc:8 * cc + 8, :]
        gest2 = np.empty((128, 2 * BPC), dtype=f32)
        for kt in range(2):
            for i, b in enumerate(bs):
                gest2[:, kt * BPC + i] = gestalt[b, kt * 128:(kt + 1) * 128]
        in_maps.append({
            "mdf": np.ascontiguousarray(mdf[bs]),
            "cxcy": cxcy, "ohxy": ohxy, "selm": selm,
            "w1a": w1a, "w1bd": w1bd, "w1g": w1g, "gest2": gest2,
            "w2s": w2s, "b1t": b1t, "b2t": b2t,
        })
    return in_maps


last_results = None  # BassKernelResults of the most recent run (for test.py)


def _enable_ldw_opt():
    """Flip walrus's --enable-ldw-opt to true so LDWEIGHTS can target the
    PE background weight buffer (weight-load / matmul overlap)."""
    import concourse.bass_utils as bu
    if getattr(bu, "_ldw_opt_patched", False):
        return
    orig = bu.run_command

    def patched(cmd, *a, **kw):
        if isinstance(cmd, list):
            cmd = ["--enable-ldw-opt=true" if c == "--enable-ldw-opt=false"
                   else c for c in cmd]
        return orig(cmd, *a, **kw)

    bu.run_command = patched
    bu._ldw_opt_patched = True


def kernel(position, gestalt, mask, depth, weight1, bias1, weight2, bias2,
           trace=False):
    global last_results
    from concourse.bass_utils import run_bass_kernel_spmd

    if "nc" not in _cache:
        _cache["nc"] = _build_nc()
    nc = _cache["nc"]

    in_maps = _host_prep(position, gestalt, mask, depth,
                         np.asarray(weight1), np.asarray(bias1),
                         np.asarray(weight2), np.asarray(bias2))
    res = run_bass_kernel_spmd(nc, in_maps, list(range(NCORES)), trace=trace)
    last_results = res

    out = np.empty((B, BASE, PH, PW), dtype=np.float32)
    for core in range(NCORES):
        o = res.results[core]["out"].reshape(BPC, BASE, PH, PW)
        out[core * BPC:(core + 1) * BPC] = o
    return out


# revision 38
# speedup vs baseline: 1.5528x; 1.2480x over previous
"""Trainium2 Bass kernel for nn_MemoryEfficientRGBDecoderStem.

Reference computation (per batch b of 16):
  - Fourier positional embedding: 32 channels of cos((grid - pos)*std*pi/2*2^f),
    separable in x / y.
  - inp = concat([gestalt (256ch) * mask, embedding (32ch) * mask, depth * mask])
  - space-to-depth 2x2 patchify -> rows (64*64 patches, 289*4=1156 cols)
  - h = silu(p @ W1.T + b1)  (hidden 1024)
  - o = h @ W2.T + b2        (base 256) -> (B, 256, 64, 64)

Strategy: data-parallel over batch, 2 batches per NeuronCore (8 cores).
The GEMM1 contraction is algebraically reduced from K=1156 to K=136:
  * the 1024 gestalt*mask columns collapse to 4 "mask" features with
    per-batch folded weights wg[s,n] = sum_c gestalt[c] * W1[n,(c,s)]
    (wg computed on-device with a small matmul),
  * the 128 cos-embedding columns are built on device via one-hot
    broadcast matmuls (cos tables x one-hot over px/py) times mask,
  * 4 depth*mask features.
All matmuls run in float32r (fast fp32 mode, 1 cycle/row at N>=512).
"""

import numpy as np

# ---- static problem shapes ----
B, G, H, W = 16, 256, 128, 128
NF, S = 16, 2
HID, BASE = 1024, 256
NCORES = 8
BPC = B // NCORES          # batches per core = 2
PH, PW = H // S, W // S    # 64, 64
R = PH * PW                # 4096 patch rows per batch
CH = 512                   # r-chunk size
NCH = R // CH              # 8 chunks per batch
NT1 = HID // 128           # 8 n-tiles for GEMM1
NT2 = BASE // 128          # 2 m-tiles for GEMM2
KT2 = HID // 128           # 8 k-tiles for GEMM2

_cache = {}


def _build_nc():
    import concourse.tile as tile
    from concourse import bacc, mybir

    dt = mybir.dt
    f32, f32r, bf16 = dt.float32, dt.float32r, dt.bfloat16

    nc = bacc.Bacc("TRN2", target_bir_lowering=False, debug=False,
                   num_devices=NCORES)

    # ---- DRAM I/O (per core) ----
    d_mdf = nc.dram_tensor("mdf", [BPC, 8, R], f32, kind="ExternalInput").ap()
    d_cxcy = nc.dram_tensor("cxcy", [72, BPC * NCH * 128], f32,
                            kind="ExternalInput").ap()
    d_ohxy = nc.dram_tensor("ohxy", [72, CH], f32, kind="ExternalInput").ap()
    d_selm = nc.dram_tensor("selm", [4, 128], f32, kind="ExternalInput").ap()
    d_w1a = nc.dram_tensor("w1a", [128, HID], f32, kind="ExternalInput").ap()
    d_w1bd = nc.dram_tensor("w1bd", [4, HID], f32, kind="ExternalInput").ap()
    d_w1g = nc.dram_tensor("w1g", [128, 2 * 4 * HID], f32, kind="ExternalInput").ap()
    d_gest = nc.dram_tensor("gest2", [128, 2 * BPC], f32, kind="ExternalInput").ap()
    d_w2s = nc.dram_tensor("w2s", [128, KT2 * BASE], f32, kind="ExternalInput").ap()
    d_b1t = nc.dram_tensor("b1t", [128, NT1], f32, kind="ExternalInput").ap()
    d_b2t = nc.dram_tensor("b2t", [128, NT2], f32, kind="ExternalInput").ap()
    d_out = nc.dram_tensor("out", [BPC, BASE, R], f32, kind="ExternalOutput").ap()

    with tile.TileContext(nc) as tc:
        with tc.tile_pool(name="const", bufs=1) as cp, \
             tc.tile_pool(name="stage", bufs=2) as stp, \
             tc.tile_pool(name="feat", bufs=3) as fp, \
             tc.tile_pool(name="h", bufs=3) as hp, \
             tc.tile_pool(name="outp", bufs=4) as op, \
             tc.tile_pool(name="ps_feat", bufs=1, space="PSUM") as psf, \
             tc.tile_pool(name="ps_cv", bufs=2, space="PSUM") as pcv, \
             tc.tile_pool(name="ps_g1", bufs=3, space="PSUM") as ps1, \
             tc.tile_pool(name="ps_g2", bufs=2, space="PSUM") as ps2:

            # ---- load + round constants ----
            def load_round(dram_ap, shape, pool=cp, rdt=f32r):
                st = stp.tile(shape, f32, tag="stage")
                nc.sync.dma_start(st[:], dram_ap)
                rt = pool.tile(shape, rdt, tag=f"r{dram_ap.tensor.name}")
                nc.vector.tensor_copy(rt[:], st[:])
                return rt

            # load order matters: everything the first chunks need comes
            # before the (large) gestalt-fold weight stream.
            selm_r = load_round(d_selm, [4, 128], rdt=bf16)
            ohxy_r = load_round(d_ohxy, [72, CH], rdt=bf16)

            # ---- per-batch mask/depth features ----
            # mdf rows 0..3 = mask_s[r], rows 4..7 = depth_s[r] -> *= mask.
            # Compute engines can only address partition bases 0/32/64/96,
            # so build the depth*mask product at base 0 and DMA it into
            # partitions 4..7 of the rounded tile.
            mdf_r = []
            with tc.tile_pool(name="mdfstp", bufs=1) as mstp:
                for b in range(BPC):
                    mkf = mstp.tile([4, R], f32, tag=f"maskst{b}")
                    nc.sync.dma_start(mkf[:], d_mdf[b, 0:4])
                    dpf = mstp.tile([4, R], f32, tag="depthst")
                    nc.sync.dma_start(dpf[:], d_mdf[b, 4:8])
                    dpr = mstp.tile([4, R], bf16, tag="depthr")
                    nc.vector.tensor_tensor(dpr[:], dpf[:], mkf[:],
                                            mybir.AluOpType.mult)
                    mr = cp.tile([8, R], bf16, tag=f"mdfr{b}")
                    nc.vector.tensor_copy(mr[0:4, :], mkf[:])
                    nc.sync.dma_start(mr[4:8, :], dpr[:])
                    mdf_r.append(mr)

            cxcy_r = load_round(d_cxcy, [72, BPC * NCH * 128], rdt=bf16)
            w1a_r = load_round(d_w1a, [128, HID], rdt=bf16)

            b1t = cp.tile([128, NT1], f32)
            nc.sync.dma_start(b1t[:], d_b1t)
            b2t = cp.tile([128, NT2], f32)
            nc.sync.dma_start(b2t[:], d_b2t)

            blhsT = []
            # transient pools for the gestalt fold; close before the main loop
            # (PSUM comes from the ps_g2 pool, which is idle until the loop)
            with tc.tile_pool(name="wg1", bufs=1) as wg1, \
                 tc.tile_pool(name="wg2", bufs=4) as wg2:
                pswg = ps2
                gest_r = load_round(d_gest, [128, 2 * BPC], pool=wg1)
                # wg[b, s*HID+n] = sum_c gestalt[b,c] W1[n,(c,s)],
                # streaming w1g in 512-wide column chunks
                wgst = wg1.tile([BPC, 4 * HID], f32, tag="wgst")
                for sub in range(4 * HID // 512):
                    ps = pswg.tile([BPC, 512], f32, tag="g2")
                    for kt in range(2):
                        wc = wg2.tile([128, 512], f32, tag="wgchunk")
                        nc.sync.dma_start(
                            wc[:],
                            d_w1g[:, kt * 4 * HID + sub * 512:
                                  kt * 4 * HID + (sub + 1) * 512])
                        wcr = wg2.tile([128, 512], f32r, tag="wgchunkr")
                        nc.vector.tensor_copy(wcr[:], wc[:])
                        nc.tensor.matmul(
                            ps[:], gest_r[:, kt * BPC:(kt + 1) * BPC], wcr[:],
                            start=(kt == 0), stop=(kt == 1))
                    nc.scalar.copy(wgst[:, sub * 512:(sub + 1) * 512], ps[:])

                # blhsT per batch: rows 0..3 = wg, rows 4..7 = depth W1.
                # Re-partition (1,4096) -> (4,1024) via a DRAM round-trip.
                with tc.tile_pool(name="wgd", bufs=1, space="DRAM") as wgd:
                    dwg = wgd.tile([BPC, 4, HID], f32, tag="dwg")
                    nc.sync.dma_start(dwg[:], wgst[:].rearrange(
                        "b (s n) -> b s n", s=4))
                    for b in range(BPC):
                        bl = wg1.tile([8, HID], f32, tag=f"blhsT{b}")
                        nc.sync.dma_start(bl[0:4, :], dwg[b])
                        nc.sync.dma_start(bl[4:8, :], d_w1bd)
                        blr = cp.tile([8, HID], bf16, tag=f"blhsTr{b}")
                        nc.vector.tensor_copy(blr[:], bl[:])
                        blhsT.append(blr)

            w2s_r = load_round(d_w2s, [128, KT2 * BASE], rdt=bf16)

            # ---- main loop over (batch, r-chunk), software-pipelined:
            # the feature build for chunk c+1 is emitted before chunk c's
            # GEMMs so the PE never waits on the DVE feature product.
            chunks = [(b, cc) for b in range(BPC) for cc in range(NCH)]

            def build_feat(b, cc, idx):
                sl = slice(cc * CH, (cc + 1) * CH)
                # mask broadcast to 128 partitions (k -> s = k%4)
                mrep_ps = psf.tile([128, CH], f32, tag="mrep",
                                   name=f"mrep_ps_{idx}")
                nc.tensor.matmul(mrep_ps[:], selm_r[:],
                                 mdf_r[b][0:4, sl], start=True, stop=True)
                mrep = fp.tile([128, CH], f32, tag="mrep_sb",
                               name=f"mrep_{idx}")
                nc.vector.tensor_copy(mrep[:], mrep_ps[:])
                # cos values broadcast over the chunk: single K=72 matmul
                # with block-diagonal lhsT [cx (64 px rows) | cy (8 py rows)]
                # against the stacked one-hot rhs (chunk-invariant pattern).
                blk = b * NCH + cc
                cv_ps = pcv.tile([128, CH], f32, tag="cv",
                                 name=f"cv_ps_{idx}")
                nc.tensor.matmul(cv_ps[:],
                                 cxcy_r[:, blk * 128:(blk + 1) * 128],
                                 ohxy_r[:], start=True, stop=True)
                # embedding features = cos * mask
                fa = fp.tile([128, CH], bf16, tag="fa", name=f"fa_{idx}")
                nc.vector.tensor_mul(fa[:], cv_ps[:], mrep[:])
                return fa

            fa_next = build_feat(*chunks[0], 0)
            for idx, (b, cc) in enumerate(chunks):
                r0 = cc * CH
                sl = slice(r0, r0 + CH)
                fa = fa_next
                if idx + 1 < len(chunks):
                    fa_next = build_feat(*chunks[idx + 1], idx + 1)

                # GEMM1 + bias1 + SiLU -> hT chunk (8 n-tiles side by
                # side). Matmuls are emitted pair-interleaved
                # (A0,A1,B0,B1,...) so consecutive matmuls never
                # accumulate into the same PSUM bank — the drain of one
                # bank hides under the fill of the other.
                if True:
                    hT = hp.tile([128, NT1 * CH], bf16, tag="hT",
                                 name=f"hT_{idx}")
                    for np_ in range(NT1 // 2):
                        nts = (2 * np_, 2 * np_ + 1)
                        g1s = []
                        for nt in nts:
                            g1 = ps1.tile([128, CH], f32, tag="g1")
                            nc.tensor.matmul(g1[:],
                                             w1a_r[:, nt * 128:(nt + 1) * 128],
                                             fa[:], start=True, stop=False)
                            g1s.append(g1)
                        for nt, g1 in zip(nts, g1s):
                            nc.tensor.matmul(g1[:],
                                             blhsT[b][:, nt * 128:(nt + 1) * 128],
                                             mdf_r[b][:, sl],
                                             start=False, stop=True)
                            nc.scalar.activation(
                                hT[:, nt * CH:(nt + 1) * CH], g1[:],
                                mybir.ActivationFunctionType.Silu,
                                bias=b1t[:, nt:nt + 1], scale=1.0)

                    # GEMM2 + bias2 -> out chunk; kt outer / mt inner so the
                    # two accumulating banks alternate every matmul.
                    g2s = [ps2.tile([128, CH], f32, tag="g2",
                                    name=f"g2_{idx}_{mt}")
                           for mt in range(NT2)]
                    for kt in range(KT2):
                        for mt in range(NT2):
                            nc.tensor.matmul(
                                g2s[mt][:],
                                w2s_r[:, kt * BASE + mt * 128:
                                      kt * BASE + (mt + 1) * 128],
                                hT[:, kt * CH:(kt + 1) * CH],
                                start=(kt == 0), stop=(kt == KT2 - 1))
                    for mt in range(NT2):
                        ob = op.tile([128, CH], f32, tag="ob")
                        nc.vector.tensor_scalar_add(ob[:], g2s[mt][:],
                                                    b2t[:, mt:mt + 1])
                        nc.sync.dma_start(
                            d_out[b, mt * 128:(mt + 1) * 128, sl], ob[:])

    nc.compile()
    return nc


def _host_prep(position, gestalt, mask, depth, weight1, bias1, weight2, bias2):
    """Pure-layout host prep + tiny cos tables. Returns per-core input maps."""
    f32 = np.float32
    # --- cos tables, replicating the reference's fp32 op order exactly ---
    gx = np.arange(W, dtype=f32)
    gx = gx / f32(W - 1)
    gx = gx * f32(2)
    gx = gx - f32(1)          # (128,) == gy since H == W
    x = np.clip(position[:, 0], f32(-1.0), f32(1.0)).astype(f32)[:, None]
    y = np.clip(position[:, 1], f32(-1.0), f32(1.0)).astype(f32)[:, None]
    min_std = f32(1.0 / min(H, W))
    std = (f32(0.1) / np.clip(position[:, 3], min_std, f32(0.5)).astype(f32))[:, None]
    half_pi = f32(np.pi / 2)
    ngx = ((gx[None, :] - x) * std) * half_pi          # (B, 128) fp32
    ngy = ((gx[None, :] - y) * std) * half_pi
    freqs = (f32(2.0) ** np.arange(NF, dtype=f32))[None, :, None]
    ax = (ngx[:, None, :] * freqs).astype(f32)         # (B, NF, 128)
    ay = (ngy[:, None, :] * freqs).astype(f32)
    cosx = np.cos(ax.astype(np.float64)).astype(f32)   # true cos of fp32 angle
    cosy = np.cos(ay.astype(np.float64)).astype(f32)

    # cxa[b, px, f*4+s] = cosx[b, f, 2px + s%2]; cya uses s//2 with py
    cxa = np.empty((B, 64, 64), dtype=f32)
    cya = np.empty((B, 64, 64), dtype=f32)
    px2 = 2 * np.arange(64)
    for s in range(4):
        sx, sy = s % 2, s // 2
        cxa[:, :, np.arange(NF) * 4 + s] = cosx[:, :, px2 + sx].transpose(0, 2, 1)
        cya[:, :, np.arange(NF) * 4 + s] = cosy[:, :, px2 + sy].transpose(0, 2, 1)

    # --- mask/depth patchify: mdf[b, 0:4, r]=mask_s, [4:8]=depth_s (raw) ---
    def patch(a):  # (B,1,H,W) -> (B, 4, R) with s=(sy*2+sx), r=py*64+px
        a6 = a[:, 0].reshape(B, PH, S, PW, S)
        return a6.transpose(0, 2, 4, 1, 3).reshape(B, 4, R).astype(f32)

    mdf = np.concatenate([patch(mask), patch(depth)], axis=1)  # (B, 8, R)

    # --- weight reshuffles ---
    w1e = weight1[:, G * 4:(G + 2 * NF) * 4].reshape(HID, 2 * NF, 4)
    w1a = np.concatenate([
        w1e[:, 0::2, :].transpose(1, 2, 0).reshape(64, HID),   # even ch (cos_x)
        w1e[:, 1::2, :].transpose(1, 2, 0).reshape(64, HID),   # odd ch (cos_y)
    ], axis=0).astype(f32)                                     # (128, HID)
    w1bd = weight1[:, (G + 2 * NF) * 4:].T.astype(f32).copy()  # (4, HID)
    w1g = weight1[:, :G * 4].reshape(HID, G, 4).transpose(1, 2, 0)  # (256,4,HID)
    w1g = w1g.reshape(2, 128, 4 * HID).transpose(1, 0, 2).reshape(128, 2 * 4 * HID)
    w1g = np.ascontiguousarray(w1g, dtype=f32)
    w2s = weight2.T.reshape(KT2, 128, BASE).transpose(1, 0, 2).reshape(128, KT2 * BASE)
    w2s = np.ascontiguousarray(w2s, dtype=f32)
    b1t = np.ascontiguousarray(bias1.reshape(NT1, 128).T, dtype=f32)
    b2t = np.ascontiguousarray(bias2.reshape(NT2, 128).T, dtype=f32)

    # stacked one-hot rhs: rows 0..63 delta(p, px), rows 64..71 delta(p, py%8)
    ohxy = np.zeros((72, CH), dtype=f32)
    ohxy[0:64] = np.tile(np.eye(64, dtype=f32), (1, CH // 64))
    ohxy[64:72] = np.repeat(np.eye(8, dtype=f32), 64, axis=1)
    selm = np.tile(np.eye(4, dtype=f32), (1, 32))           # delta(s, k%4)

    in_maps = []
    for core in range(NCORES):
        bs = [core * BPC + i for i in range(BPC)]
        # block-diagonal cos lhsT per (batch, chunk): cols 0..63 are the 64
        # cos_x features (rows = px slots), cols 64..127 the cos_y features
        # (rows 64..71 = the chunk's 8 py slots)
        cxcy = np.zeros((72, BPC * NCH * 128), dtype=f32)
        for i, b in enumerate(bs):
            for cc in range(NCH):
                c0 = (i * NCH + cc) * 128
                cxcy[0:64, c0:c0 + 64] = cxa[b]
                cxcy[64:72, c0 + 64:c0 + 128] = cya[b, 8 * cc:8 * cc + 8, :]
        gest2 = np.empty((128, 2 * BPC), dtype=f32)
        for kt in range(2):
            for i, b in enumerate(bs):
                gest2[:, kt * BPC + i] = gestalt[b, kt * 128:(kt + 1) * 128]
        in_maps.append({
            "mdf": np.ascontiguousarray(mdf[bs]),
            "cxcy": cxcy, "ohxy": ohxy, "selm": selm,
            "w1a": w1a, "w1bd": w1bd, "w1g": w1g, "gest2": gest2,
            "w2s": w2s, "b1t": b1t, "b2t": b2t,
        })
    return in_maps


last_results = None  # BassKernelResults of the most recent run (for test.py)


def _enable_ldw_opt():
    """Flip walrus's --enable-ldw-opt to true so LDWEIGHTS can target the
    PE background weight buffer (weight-load / matmul overlap)."""
    import concourse.bass_utils as bu
    if getattr(bu, "_ldw_opt_patched", False):
        return
    orig = bu.run_command

    def patched(cmd, *a, **kw):
        if isinstance(cmd, list):
            cmd = ["--enable-ldw-opt=true" if c == "--enable-ldw-opt=false"
                   else c for c in cmd]
        return orig(cmd, *a, **kw)

    bu.run_command = patched
    bu._ldw_opt_patched = True


def kernel(position, gestalt, mask, depth, weight1, bias1, weight2, bias2,
           trace=False):
    global last_results
    from concourse.bass_utils import run_bass_kernel_spmd

    if "nc" not in _cache:
        _cache["nc"] = _build_nc()
    nc = _cache["nc"]

    in_maps = _host_prep(position, gestalt, mask, depth,
                         np.asarray(weight1), np.asarray(bias1),
                         np.asarray(weight2), np.asarray(bias2))
    res = run_bass_kernel_spmd(nc, in_maps, list(range(NCORES)), trace=trace)
    last_results = res

    out = np.empty((B, BASE, PH, PW), dtype=np.float32)
    for core in range(NCORES):
        o = res.results[core]["out"].reshape(BPC, BASE, PH, PW)
        out[core * BPC:(core + 1) * BPC] = o
    return out


# revision 40
# speedup vs baseline: 1.6898x; 1.0883x over previous
"""Trainium2 Bass kernel for nn_MemoryEfficientRGBDecoderStem.

Reference computation (per batch b of 16):
  - Fourier positional embedding: 32 channels of cos((grid - pos)*std*pi/2*2^f),
    separable in x / y.
  - inp = concat([gestalt (256ch) * mask, embedding (32ch) * mask, depth * mask])
  - space-to-depth 2x2 patchify -> rows (64*64 patches, 289*4=1156 cols)
  - h = silu(p @ W1.T + b1)  (hidden 1024)
  - o = h @ W2.T + b2        (base 256) -> (B, 256, 64, 64)

Strategy: data-parallel over batch, 2 batches per NeuronCore (8 cores).
The GEMM1 contraction is algebraically reduced from K=1156 to K=136:
  * the 1024 gestalt*mask columns collapse to 4 "mask" features with
    per-batch folded weights wg[s,n] = sum_c gestalt[c] * W1[n,(c,s)]
    (wg computed on-device with a small matmul),
  * the 128 cos-embedding columns are built on device via one-hot
    broadcast matmuls (cos tables x one-hot over px/py) times mask,
  * 4 depth*mask features.
All matmuls run in float32r (fast fp32 mode, 1 cycle/row at N>=512).
"""

import numpy as np

# ---- static problem shapes ----
B, G, H, W = 16, 256, 128, 128
NF, S = 16, 2
HID, BASE = 1024, 256
NCORES = 8
BPC = B // NCORES          # batches per core = 2
PH, PW = H // S, W // S    # 64, 64
R = PH * PW                # 4096 patch rows per batch
CH = 512                   # r-chunk size
NCH = R // CH              # 8 chunks per batch
NT1 = HID // 128           # 8 n-tiles for GEMM1
NT2 = BASE // 128          # 2 m-tiles for GEMM2
KT2 = HID // 128           # 8 k-tiles for GEMM2

_cache = {}


def _build_nc():
    import concourse.tile as tile
    from concourse import bacc, mybir

    dt = mybir.dt
    f32, f16 = dt.float32, dt.float16

    nc = bacc.Bacc("TRN2", target_bir_lowering=False, debug=False,
                   num_devices=NCORES)

    # ---- DRAM I/O (per core); matmul operands are host-prepped fp16 ----
    d_mdf = nc.dram_tensor("mdf", [BPC, 9, R], f16, kind="ExternalInput").ap()
    d_cxcy = nc.dram_tensor("cxcy", [72, BPC * NCH * 128], f16,
                            kind="ExternalInput").ap()
    d_ohxy = nc.dram_tensor("ohxy", [72, CH], f16, kind="ExternalInput").ap()
    d_selm = nc.dram_tensor("selm", [4, 128], f16, kind="ExternalInput").ap()
    d_w1a = nc.dram_tensor("w1a", [128, HID], f16, kind="ExternalInput").ap()
    # w1bd5: rows 0..3 depth weights, row 4 = bias1 (folded into the GEMM)
    d_w1bd = nc.dram_tensor("w1bd", [5, HID], f32, kind="ExternalInput").ap()
    d_w1g = nc.dram_tensor("w1g", [128, 2 * 4 * HID], f16,
                           kind="ExternalInput").ap()
    d_gest = nc.dram_tensor("gest2", [128, 2 * BPC], f16,
                            kind="ExternalInput").ap()
    d_w2s = nc.dram_tensor("w2s", [128, KT2 * BASE], f16,
                           kind="ExternalInput").ap()
    d_b2t = nc.dram_tensor("b2t", [128, NT2], f32, kind="ExternalInput").ap()
    d_out = nc.dram_tensor("out", [BPC, BASE, R], f32, kind="ExternalOutput").ap()

    with tile.TileContext(nc) as tc:
        with tc.tile_pool(name="const", bufs=1) as cp, \
             tc.tile_pool(name="feat", bufs=3) as fp, \
             tc.tile_pool(name="h", bufs=3) as hp, \
             tc.tile_pool(name="outp", bufs=4) as op, \
             tc.tile_pool(name="ps_mrep", bufs=1, space="PSUM") as psf, \
             tc.tile_pool(name="ps_cv", bufs=1, space="PSUM") as pcv, \
             tc.tile_pool(name="ps_g1", bufs=2, space="PSUM") as ps1, \
             tc.tile_pool(name="ps_g2", bufs=2, space="PSUM") as ps2:

            def load(dram_ap, shape, dtype=f16, pool=cp):
                t = pool.tile(shape, dtype, tag=f"t{dram_ap.tensor.name}")
                nc.sync.dma_start(t[:], dram_ap)
                return t

            # load order matters: everything the first chunks need comes
            # before the (large) gestalt-fold weight stream.
            selm_h = load(d_selm, [4, 128])
            ohxy_h = load(d_ohxy, [72, CH])

            # per-batch feature rows: 0..3 mask_s, 4..7 depth_s*mask_s,
            # 8 ones (bias path). Compute engines only address partition
            # bases 0/32/64/96, so the depth*mask product is built at base 0
            # and DMA'd into partitions 4..8.
            mdf_r = []
            with tc.tile_pool(name="mdfstp", bufs=1) as mstp:
                for b in range(BPC):
                    mkf = mstp.tile([4, R], f16, tag=f"maskst{b}")
                    nc.sync.dma_start(mkf[:], d_mdf[b, 0:4])
                    dpf = mstp.tile([4, R], f16, tag="depthst")
                    nc.sync.dma_start(dpf[:], d_mdf[b, 4:8])
                    dpr = mstp.tile([4, R], f16, tag="depthr")
                    nc.vector.tensor_tensor(dpr[:], dpf[:], mkf[:],
                                            mybir.AluOpType.mult)
                    mr = cp.tile([9, R], f16, tag=f"mdfr{b}")
                    nc.vector.tensor_copy(mr[0:4, :], mkf[:])
                    nc.sync.dma_start(mr[4:8, :], dpr[:])
                    nc.sync.dma_start(mr[8:9, :], d_mdf[b, 8:9])
                    mdf_r.append(mr)

            cxcy_h = load(d_cxcy, [72, BPC * NCH * 128])
            w1a_h = load(d_w1a, [128, HID])
            w2s_h = load(d_w2s, [128, KT2 * BASE])
            b2t = load(d_b2t, [128, NT2], dtype=f32)

            # ---- gestalt fold: wg[b, s*HID+n] = sum_c gestalt[b,c] W1[n,(c,s)]
            blhsT = []
            with tc.tile_pool(name="wg1", bufs=1) as wg1:
                gest_h = load(d_gest, [128, 2 * BPC], pool=wg1)
                w1g_h = load(d_w1g, [128, 2 * 4 * HID], pool=wg1)
                wgst = wg1.tile([BPC, 4 * HID], f32, tag="wgst")
                for sub in range(4 * HID // 512):
                    ps = ps2.tile([BPC, 512], f32, tag="g2",
                                  name=f"wgps_{sub}")
                    for kt in range(2):
                        nc.tensor.matmul(
                            ps[:], gest_h[:, kt * BPC:(kt + 1) * BPC],
                            w1g_h[:, kt * 4 * HID + sub * 512:
                                  kt * 4 * HID + (sub + 1) * 512],
                            start=(kt == 0), stop=(kt == 1))
                    nc.scalar.copy(wgst[:, sub * 512:(sub + 1) * 512], ps[:])

                # blhsT per batch (9, HID): rows 0..3 wg, 4..7 depth W1,
                # row 8 bias1. Re-partition (1,4096) -> (4,1024) via DRAM.
                with tc.tile_pool(name="wgd", bufs=1, space="DRAM") as wgd:
                    dwg = wgd.tile([BPC, 4, HID], f32, tag="dwg")
                    nc.sync.dma_start(dwg[:], wgst[:].rearrange(
                        "b (s n) -> b s n", s=4))
                    for b in range(BPC):
                        bl = wg1.tile([9, HID], f32, tag=f"blhsT{b}")
                        nc.sync.dma_start(bl[0:4, :], dwg[b])
                        nc.sync.dma_start(bl[4:9, :], d_w1bd)
                        blr = cp.tile([9, HID], f16, tag=f"blhsTr{b}")
                        nc.vector.tensor_copy(blr[:], bl[:])
                        blhsT.append(blr)

            # ---- main loop over (batch, r-chunk), software-pipelined:
            # the feature build for chunk c+1 is emitted before chunk c's
            # GEMMs so the PE never waits on the DVE feature product.
            chunks = [(b, cc) for b in range(BPC) for cc in range(NCH)]

            def build_feat(b, cc, idx):
                sl = slice(cc * CH, (cc + 1) * CH)
                # mask broadcast to 128 partitions (k -> s = k%4)
                mrep_ps = psf.tile([128, CH], f32, tag="mrep",
                                   name=f"mrep_ps_{idx}")
                nc.tensor.matmul(mrep_ps[:], selm_h[:],
                                 mdf_r[b][0:4, sl], start=True, stop=True)
                mrep = fp.tile([128, CH], f32, tag="mrep_sb",
                               name=f"mrep_{idx}")
                nc.vector.tensor_copy(mrep[:], mrep_ps[:])
                # cos values broadcast over the chunk: single K=72 matmul
                # with block-diagonal lhsT [cx (64 px rows) | cy (8 py rows)]
                # against the stacked one-hot rhs (chunk-invariant pattern).
                blk = b * NCH + cc
                cv_ps = pcv.tile([128, CH], f32, tag="cv",
                                 name=f"cv_ps_{idx}")
                nc.tensor.matmul(cv_ps[:],
                                 cxcy_h[:, blk * 128:(blk + 1) * 128],
                                 ohxy_h[:], start=True, stop=True)
                # embedding features = cos * mask
                fa = fp.tile([128, CH], f16, tag="fa", name=f"fa_{idx}")
                nc.vector.tensor_mul(fa[:], cv_ps[:], mrep[:])
                return fa

            fa_next = build_feat(*chunks[0], 0)
            for idx, (b, cc) in enumerate(chunks):
                r0 = cc * CH
                sl = slice(r0, r0 + CH)
                fa = fa_next
                if idx + 1 < len(chunks):
                    fa_next = build_feat(*chunks[idx + 1], idx + 1)

                # GEMM1 (+bias1 via the ones-feature) + SiLU -> hT chunk.
                # n-tiles processed in pairs sharing one 2-bank PSUM tile;
                # matmuls pair-interleaved (A_e, A_o, B_e, B_o) so
                # consecutive matmuls hit different banks, and one SiLU
                # covers the pair.
                hT = hp.tile([128, NT1 * CH], f16, tag="hT",
                             name=f"hT_{idx}")
                for np_ in range(NT1 // 2):
                    g1 = ps1.tile([128, 2 * CH], f32, tag="g1",
                                  name=f"g1_{idx}_{np_}")
                    for half in range(2):
                        nt = 2 * np_ + half
                        nc.tensor.matmul(
                            g1[:, half * CH:(half + 1) * CH],
                            w1a_h[:, nt * 128:(nt + 1) * 128],
                            fa[:], start=True, stop=False)
                    for half in range(2):
                        nt = 2 * np_ + half
                        nc.tensor.matmul(
                            g1[:, half * CH:(half + 1) * CH],
                            blhsT[b][:, nt * 128:(nt + 1) * 128],
                            mdf_r[b][:, sl], start=False, stop=True)
                    nc.scalar.activation(
                        hT[:, 2 * np_ * CH:(2 * np_ + 2) * CH], g1[:],
                        mybir.ActivationFunctionType.Silu)

                # GEMM2 + bias2 -> out chunk; kt outer / mt inner so the
                # two accumulating banks alternate every matmul.
                g2s = [ps2.tile([128, CH], f32, tag="g2",
                                name=f"g2_{idx}_{mt}")
                       for mt in range(NT2)]
                for kt in range(KT2):
                    for mt in range(NT2):
                        nc.tensor.matmul(
                            g2s[mt][:],
                            w2s_h[:, kt * BASE + mt * 128:
                                  kt * BASE + (mt + 1) * 128],
                            hT[:, kt * CH:(kt + 1) * CH],
                            start=(kt == 0), stop=(kt == KT2 - 1))
                for mt in range(NT2):
                    ob = op.tile([128, CH], f32, tag="ob",
                                 name=f"ob_{idx}_{mt}")
                    nc.vector.tensor_scalar_add(ob[:], g2s[mt][:],
                                                b2t[:, mt:mt + 1])
                    nc.sync.dma_start(
                        d_out[b, mt * 128:(mt + 1) * 128, sl], ob[:])

    nc.compile()
    return nc


def _host_prep(position, gestalt, mask, depth, weight1, bias1, weight2, bias2):
    """Pure-layout host prep + tiny cos tables. Returns per-core input maps."""
    f32 = np.float32
    # --- cos tables, replicating the reference's fp32 op order exactly ---
    gx = np.arange(W, dtype=f32)
    gx = gx / f32(W - 1)
    gx = gx * f32(2)
    gx = gx - f32(1)          # (128,) == gy since H == W
    x = np.clip(position[:, 0], f32(-1.0), f32(1.0)).astype(f32)[:, None]
    y = np.clip(position[:, 1], f32(-1.0), f32(1.0)).astype(f32)[:, None]
    min_std = f32(1.0 / min(H, W))
    std = (f32(0.1) / np.clip(position[:, 3], min_std, f32(0.5)).astype(f32))[:, None]
    half_pi = f32(np.pi / 2)
    ngx = ((gx[None, :] - x) * std) * half_pi          # (B, 128) fp32
    ngy = ((gx[None, :] - y) * std) * half_pi
    freqs = (f32(2.0) ** np.arange(NF, dtype=f32))[None, :, None]
    ax = (ngx[:, None, :] * freqs).astype(f32)         # (B, NF, 128)
    ay = (ngy[:, None, :] * freqs).astype(f32)
    cosx = np.cos(ax.astype(np.float64)).astype(f32)   # true cos of fp32 angle
    cosy = np.cos(ay.astype(np.float64)).astype(f32)

    # cxa[b, px, f*4+s] = cosx[b, f, 2px + s%2]; cya uses s//2 with py
    cxa = np.empty((B, 64, 64), dtype=f32)
    cya = np.empty((B, 64, 64), dtype=f32)
    px2 = 2 * np.arange(64)
    for s in range(4):
        sx, sy = s % 2, s // 2
        cxa[:, :, np.arange(NF) * 4 + s] = cosx[:, :, px2 + sx].transpose(0, 2, 1)
        cya[:, :, np.arange(NF) * 4 + s] = cosy[:, :, px2 + sy].transpose(0, 2, 1)

    # --- mask/depth patchify: mdf[b, 0:4, r]=mask_s, [4:8]=depth_s (raw) ---
    def patch(a):  # (B,1,H,W) -> (B, 4, R) with s=(sy*2+sx), r=py*64+px
        a6 = a[:, 0].reshape(B, PH, S, PW, S)
        return a6.transpose(0, 2, 4, 1, 3).reshape(B, 4, R).astype(f32)

    f16 = np.float16
    # mdf rows: 0..3 mask_s, 4..7 depth_s (raw), 8 ones (bias-1 feature)
    mdf = np.concatenate(
        [patch(mask), patch(depth), np.ones((B, 1, R), dtype=f32)],
        axis=1).astype(f16)                                    # (B, 9, R)

    # --- weight reshuffles ---
    w1e = weight1[:, G * 4:(G + 2 * NF) * 4].reshape(HID, 2 * NF, 4)
    w1a = np.concatenate([
        w1e[:, 0::2, :].transpose(1, 2, 0).reshape(64, HID),   # even ch (cos_x)
        w1e[:, 1::2, :].transpose(1, 2, 0).reshape(64, HID),   # odd ch (cos_y)
    ], axis=0).astype(f16)                                     # (128, HID)
    # w1bd: rows 0..3 depth weights, row 4 = bias1
    w1bd = np.concatenate(
        [weight1[:, (G + 2 * NF) * 4:].T, bias1[None, :]], axis=0).astype(f32)
    w1g = weight1[:, :G * 4].reshape(HID, G, 4).transpose(1, 2, 0)  # (256,4,HID)
    w1g = w1g.reshape(2, 128, 4 * HID).transpose(1, 0, 2).reshape(128, 2 * 4 * HID)
    w1g = np.ascontiguousarray(w1g, dtype=f16)
    w2s = weight2.T.reshape(KT2, 128, BASE).transpose(1, 0, 2).reshape(128, KT2 * BASE)
    w2s = np.ascontiguousarray(w2s, dtype=f16)
    b2t = np.ascontiguousarray(bias2.reshape(NT2, 128).T, dtype=f32)

    # stacked one-hot rhs: rows 0..63 delta(p, px), rows 64..71 delta(p, py%8)
    ohxy = np.zeros((72, CH), dtype=f16)
    ohxy[0:64] = np.tile(np.eye(64, dtype=f16), (1, CH // 64))
    ohxy[64:72] = np.repeat(np.eye(8, dtype=f16), 64, axis=1)
    selm = np.tile(np.eye(4, dtype=f16), (1, 32))           # delta(s, k%4)

    in_maps = []
    for core in range(NCORES):
        bs = [core * BPC + i for i in range(BPC)]
        # block-diagonal cos lhsT per (batch, chunk): cols 0..63 are the 64
        # cos_x features (rows = px slots), cols 64..127 the cos_y features
        # (rows 64..71 = the chunk's 8 py slots)
        cxcy = np.zeros((72, BPC * NCH * 128), dtype=f16)
        for i, b in enumerate(bs):
            for cc in range(NCH):
                c0 = (i * NCH + cc) * 128
                cxcy[0:64, c0:c0 + 64] = cxa[b]
                cxcy[64:72, c0 + 64:c0 + 128] = cya[b, 8 * cc:8 * cc + 8, :]
        gest2 = np.empty((128, 2 * BPC), dtype=f16)
        for kt in range(2):
            for i, b in enumerate(bs):
                gest2[:, kt * BPC + i] = gestalt[b, kt * 128:(kt + 1) * 128]
        in_maps.append({
            "mdf": np.ascontiguousarray(mdf[bs]),
            "cxcy": cxcy, "ohxy": ohxy, "selm": selm,
            "w1a": w1a, "w1bd": w1bd, "w1g": w1g, "gest2": gest2,
            "w2s": w2s, "b2t": b2t,
        })
    return in_maps


last_results = None  # BassKernelResults of the most recent run (for test.py)


def _enable_ldw_opt():
    """Flip walrus's --enable-ldw-opt to true so LDWEIGHTS can target the
    PE background weight buffer (weight-load / matmul overlap)."""
    import concourse.bass_utils as bu
    if getattr(bu, "_ldw_opt_patched", False):
        return
    orig = bu.run_command

    def patched(cmd, *a, **kw):
        if isinstance(cmd, list):
            cmd = ["--enable-ldw-opt=true" if c == "--enable-ldw-opt=false"
                   else c for c in cmd]
        return orig(cmd, *a, **kw)

    bu.run_command = patched
    bu._ldw_opt_patched = True


def kernel(position, gestalt, mask, depth, weight1, bias1, weight2, bias2,
           trace=False):
    global last_results
    from concourse.bass_utils import run_bass_kernel_spmd

    if "nc" not in _cache:
        _cache["nc"] = _build_nc()
    nc = _cache["nc"]

    in_maps = _host_prep(position, gestalt, mask, depth,
                         np.asarray(weight1), np.asarray(bias1),
                         np.asarray(weight2), np.asarray(bias2))
    res = run_bass_kernel_spmd(nc, in_maps, list(range(NCORES)), trace=trace)
    last_results = res

    out = np.empty((B, BASE, PH, PW), dtype=np.float32)
    for core in range(NCORES):
        o = res.results[core]["out"].reshape(BPC, BASE, PH, PW)
        out[core * BPC:(core + 1) * BPC] = o
    return out


# revision 43
# speedup vs baseline: 1.7367x; 1.0278x over previous
"""Trainium2 Bass kernel for nn_MemoryEfficientRGBDecoderStem.

Reference computation (per batch b of 16):
  - Fourier positional embedding: 32 channels of cos((grid - pos)*std*pi/2*2^f),
    separable in x / y.
  - inp = concat([gestalt (256ch) * mask, embedding (32ch) * mask, depth * mask])
  - space-to-depth 2x2 patchify -> rows (64*64 patches, 289*4=1156 cols)
  - h = silu(p @ W1.T + b1)  (hidden 1024)
  - o = h @ W2.T + b2        (base 256) -> (B, 256, 64, 64)

Strategy: data-parallel over batch, 2 batches per NeuronCore (8 cores).
The GEMM1 contraction is algebraically reduced from K=1156 to K=136:
  * the 1024 gestalt*mask columns collapse to 4 "mask" features with
    per-batch folded weights wg[s,n] = sum_c gestalt[c] * W1[n,(c,s)]
    (wg computed on-device with a small matmul),
  * the 128 cos-embedding columns are built on device via one-hot
    broadcast matmuls (cos tables x one-hot over px/py) times mask,
  * 4 depth*mask features.
All matmuls run in float32r (fast fp32 mode, 1 cycle/row at N>=512).
"""

import numpy as np

# ---- static problem shapes ----
B, G, H, W = 16, 256, 128, 128
NF, S = 16, 2
HID, BASE = 1024, 256
NCORES = 8
BPC = B // NCORES          # batches per core = 2
PH, PW = H // S, W // S    # 64, 64
R = PH * PW                # 4096 patch rows per batch
CH = 512                   # r-chunk size
NCH = R // CH              # 8 chunks per batch
NT1 = HID // 128           # 8 n-tiles for GEMM1
NT2 = BASE // 128          # 2 m-tiles for GEMM2
KT2 = HID // 128           # 8 k-tiles for GEMM2

_cache = {}


def _build_nc():
    import concourse.tile as tile
    from concourse import bacc, mybir

    dt = mybir.dt
    f32, f16 = dt.float32, dt.float16

    nc = bacc.Bacc("TRN2", target_bir_lowering=False, debug=False,
                   num_devices=NCORES)

    # ---- DRAM I/O (per core); matmul operands are host-prepped fp16 ----
    d_mdf = nc.dram_tensor("mdf", [BPC, 9, R], f16, kind="ExternalInput").ap()
    d_cxcy = nc.dram_tensor("cxcy", [72, BPC * NCH * 128], f16,
                            kind="ExternalInput").ap()
    d_ohxy = nc.dram_tensor("ohxy", [72, CH], f16, kind="ExternalInput").ap()
    d_selm = nc.dram_tensor("selm", [4, 128], f16, kind="ExternalInput").ap()
    d_w1a = nc.dram_tensor("w1a", [128, HID], f16, kind="ExternalInput").ap()
    # w1bd5: rows 0..3 depth weights, row 4 = bias1 (folded into the GEMM)
    d_w1bd = nc.dram_tensor("w1bd", [5, HID], f32, kind="ExternalInput").ap()
    d_w1g = nc.dram_tensor("w1g", [128, 2 * 4 * HID], f16,
                           kind="ExternalInput").ap()
    d_gest = nc.dram_tensor("gest2", [128, 2 * BPC], f16,
                            kind="ExternalInput").ap()
    d_w2s = nc.dram_tensor("w2s", [128, KT2 * BASE], f16,
                           kind="ExternalInput").ap()
    d_b2t = nc.dram_tensor("b2t", [128, NT2], f32, kind="ExternalInput").ap()
    d_out = nc.dram_tensor("out", [BPC, BASE, R], f32, kind="ExternalOutput").ap()

    with tile.TileContext(nc) as tc:
        with tc.tile_pool(name="const", bufs=1) as cp, \
             tc.tile_pool(name="feat", bufs=3) as fp, \
             tc.tile_pool(name="h", bufs=3) as hp, \
             tc.tile_pool(name="outp", bufs=4) as op, \
             tc.tile_pool(name="ps_mrep", bufs=1, space="PSUM") as psf, \
             tc.tile_pool(name="ps_cv", bufs=1, space="PSUM") as pcv, \
             tc.tile_pool(name="ps_g1", bufs=2, space="PSUM") as ps1, \
             tc.tile_pool(name="ps_g2", bufs=2, space="PSUM") as ps2:

            def load(dram_ap, shape, dtype=f16, pool=cp):
                t = pool.tile(shape, dtype, tag=f"t{dram_ap.tensor.name}")
                nc.sync.dma_start(t[:], dram_ap)
                return t

            # load order matters: everything the first chunks need comes
            # before the (large) gestalt-fold weight stream.
            selm_h = load(d_selm, [4, 128])
            ohxy_h = load(d_ohxy, [72, CH])

            # per-batch feature rows: 0..3 mask_s, 4..7 depth_s*mask_s,
            # 8 ones (bias path). Compute engines only address partition
            # bases 0/32/64/96, so the depth*mask product is built at base 0
            # and DMA'd into partitions 4..8.
            mdf_r = []
            with tc.tile_pool(name="mdfstp", bufs=1) as mstp:
                for b in range(BPC):
                    mkf = mstp.tile([4, R], f16, tag=f"maskst{b}")
                    nc.sync.dma_start(mkf[:], d_mdf[b, 0:4])
                    dpf = mstp.tile([4, R], f16, tag="depthst")
                    nc.sync.dma_start(dpf[:], d_mdf[b, 4:8])
                    dpr = mstp.tile([4, R], f16, tag="depthr")
                    nc.vector.tensor_tensor(dpr[:], dpf[:], mkf[:],
                                            mybir.AluOpType.mult)
                    mr = cp.tile([9, R], f16, tag=f"mdfr{b}")
                    nc.vector.tensor_copy(mr[0:4, :], mkf[:])
                    nc.sync.dma_start(mr[4:8, :], dpr[:])
                    nc.sync.dma_start(mr[8:9, :], d_mdf[b, 8:9])
                    mdf_r.append(mr)

            cxcy_h = load(d_cxcy, [72, BPC * NCH * 128])
            w1a_h = load(d_w1a, [128, HID])

            chunks = [(b, cc) for b in range(BPC) for cc in range(NCH)]

            def build_feat(b, cc, idx):
                sl = slice(cc * CH, (cc + 1) * CH)
                # mask broadcast to 128 partitions (k -> s = k%4)
                mrep_ps = psf.tile([128, CH], f32, tag="mrep",
                                   name=f"mrep_ps_{idx}")
                nc.tensor.matmul(mrep_ps[:], selm_h[:],
                                 mdf_r[b][0:4, sl], start=True, stop=True)
                mrep = fp.tile([128, CH], f32, tag="mrep_sb",
                               name=f"mrep_{idx}")
                nc.vector.tensor_copy(mrep[:], mrep_ps[:])
                # cos values broadcast over the chunk: single K=72 matmul
                # with block-diagonal lhsT [cx (64 px rows) | cy (8 py rows)]
                # against the stacked one-hot rhs (chunk-invariant pattern).
                blk = b * NCH + cc
                cv_ps = pcv.tile([128, CH], f32, tag="cv",
                                 name=f"cv_ps_{idx}")
                nc.tensor.matmul(cv_ps[:],
                                 cxcy_h[:, blk * 128:(blk + 1) * 128],
                                 ohxy_h[:], start=True, stop=True)
                # embedding features = cos * mask
                fa = fp.tile([128, CH], f16, tag="fa", name=f"fa_{idx}")
                nc.vector.tensor_mul(fa[:], cv_ps[:], mrep[:])
                return fa

            # two feature chunks emitted ahead so the PE has work while the
            # gestalt-fold weights stream in
            fa_q = [build_feat(*chunks[0], 0), build_feat(*chunks[1], 1)]

            # ---- gestalt fold: wg[b, s*HID+n] = sum_c gestalt[b,c] W1[n,(c,s)]
            # w2s loads after the fold weights (first needed later).
            blhsT = []
            with tc.tile_pool(name="wg1", bufs=1) as wg1:
                gest_h = load(d_gest, [128, 2 * BPC], pool=wg1)
                # stream w1g as 4 separately-DMA'd pieces so fold matmuls
                # start as soon as the first piece lands
                w1g_p = []
                for pc in range(4):
                    wp = wg1.tile([128, 2 * HID], f16, tag=f"w1gp{pc}",
                                  name=f"w1gp{pc}")
                    nc.sync.dma_start(
                        wp[:], d_w1g[:, pc * 2 * HID:(pc + 1) * 2 * HID])
                    w1g_p.append(wp)
                wgst = wg1.tile([BPC, 4 * HID], f32, tag="wgst")
                for sub in range(4 * HID // 512):
                    ps = ps2.tile([BPC, 512], f32, tag="g2",
                                  name=f"wgps_{sub}")
                    for kt in range(2):
                        col = kt * 4 * HID + sub * 512
                        pc, off = col // (2 * HID), col % (2 * HID)
                        nc.tensor.matmul(
                            ps[:], gest_h[:, kt * BPC:(kt + 1) * BPC],
                            w1g_p[pc][:, off:off + 512],
                            start=(kt == 0), stop=(kt == 1))
                    nc.scalar.copy(wgst[:, sub * 512:(sub + 1) * 512], ps[:])

                # blhsT per batch (9, HID): rows 0..3 wg, 4..7 depth W1,
                # row 8 bias1. Re-partition (1,4096) -> (4,1024) via DRAM.
                with tc.tile_pool(name="wgd", bufs=1, space="DRAM") as wgd:
                    dwg = wgd.tile([BPC, 4, HID], f32, tag="dwg")
                    nc.sync.dma_start(dwg[:], wgst[:].rearrange(
                        "b (s n) -> b s n", s=4))
                    for b in range(BPC):
                        bl = wg1.tile([9, HID], f32, tag=f"blhsT{b}")
                        nc.sync.dma_start(bl[0:4, :], dwg[b])
                        nc.sync.dma_start(bl[4:9, :], d_w1bd)
                        blr = cp.tile([9, HID], f16, tag=f"blhsTr{b}")
                        nc.vector.tensor_copy(blr[:], bl[:])
                        blhsT.append(blr)

            w2s_h = load(d_w2s, [128, KT2 * BASE])
            b2t = load(d_b2t, [128, NT2], dtype=f32)

            # ---- main loop, software-pipelined two chunks ahead ----
            for idx, (b, cc) in enumerate(chunks):
                r0 = cc * CH
                sl = slice(r0, r0 + CH)
                fa = fa_q.pop(0)
                if idx + 2 < len(chunks):
                    fa_q.append(build_feat(*chunks[idx + 2], idx + 2))

                # GEMM1 (+bias1 via the ones-feature) + SiLU -> hT chunk.
                # n-tiles processed in pairs sharing one 2-bank PSUM tile;
                # matmuls pair-interleaved (A_e, A_o, B_e, B_o) so
                # consecutive matmuls hit different banks, and one SiLU
                # covers the pair.
                hT = hp.tile([128, NT1 * CH], f16, tag="hT",
                             name=f"hT_{idx}")
                for np_ in range(NT1 // 2):
                    g1 = ps1.tile([128, 2 * CH], f32, tag="g1",
                                  name=f"g1_{idx}_{np_}")
                    for half in range(2):
                        nt = 2 * np_ + half
                        nc.tensor.matmul(
                            g1[:, half * CH:(half + 1) * CH],
                            w1a_h[:, nt * 128:(nt + 1) * 128],
                            fa[:], start=True, stop=False)
                    for half in range(2):
                        nt = 2 * np_ + half
                        nc.tensor.matmul(
                            g1[:, half * CH:(half + 1) * CH],
                            blhsT[b][:, nt * 128:(nt + 1) * 128],
                            mdf_r[b][:, sl], start=False, stop=True)
                    nc.scalar.activation(
                        hT[:, 2 * np_ * CH:(2 * np_ + 2) * CH], g1[:],
                        mybir.ActivationFunctionType.Silu)

                # GEMM2 + bias2 -> out chunk; kt outer / mt inner so the
                # two accumulating banks alternate every matmul.
                g2s = [ps2.tile([128, CH], f32, tag="g2",
                                name=f"g2_{idx}_{mt}")
                       for mt in range(NT2)]
                for kt in range(KT2):
                    for mt in range(NT2):
                        nc.tensor.matmul(
                            g2s[mt][:],
                            w2s_h[:, kt * BASE + mt * 128:
                                  kt * BASE + (mt + 1) * 128],
                            hT[:, kt * CH:(kt + 1) * CH],
                            start=(kt == 0), stop=(kt == KT2 - 1))
                for mt in range(NT2):
                    ob = op.tile([128, CH], f32, tag="ob",
                                 name=f"ob_{idx}_{mt}")
                    nc.vector.tensor_scalar_add(ob[:], g2s[mt][:],
                                                b2t[:, mt:mt + 1])
                    nc.sync.dma_start(
                        d_out[b, mt * 128:(mt + 1) * 128, sl], ob[:])

    nc.compile()
    return nc


def _host_prep(position, gestalt, mask, depth, weight1, bias1, weight2, bias2):
    """Pure-layout host prep + tiny cos tables. Returns per-core input maps."""
    f32 = np.float32
    # --- cos tables, replicating the reference's fp32 op order exactly ---
    gx = np.arange(W, dtype=f32)
    gx = gx / f32(W - 1)
    gx = gx * f32(2)
    gx = gx - f32(1)          # (128,) == gy since H == W
    x = np.clip(position[:, 0], f32(-1.0), f32(1.0)).astype(f32)[:, None]
    y = np.clip(position[:, 1], f32(-1.0), f32(1.0)).astype(f32)[:, None]
    min_std = f32(1.0 / min(H, W))
    std = (f32(0.1) / np.clip(position[:, 3], min_std, f32(0.5)).astype(f32))[:, None]
    half_pi = f32(np.pi / 2)
    ngx = ((gx[None, :] - x) * std) * half_pi          # (B, 128) fp32
    ngy = ((gx[None, :] - y) * std) * half_pi
    freqs = (f32(2.0) ** np.arange(NF, dtype=f32))[None, :, None]
    ax = (ngx[:, None, :] * freqs).astype(f32)         # (B, NF, 128)
    ay = (ngy[:, None, :] * freqs).astype(f32)
    cosx = np.cos(ax.astype(np.float64)).astype(f32)   # true cos of fp32 angle
    cosy = np.cos(ay.astype(np.float64)).astype(f32)

    # cxa[b, px, f*4+s] = cosx[b, f, 2px + s%2]; cya uses s//2 with py
    cxa = np.empty((B, 64, 64), dtype=f32)
    cya = np.empty((B, 64, 64), dtype=f32)
    px2 = 2 * np.arange(64)
    for s in range(4):
        sx, sy = s % 2, s // 2
        cxa[:, :, np.arange(NF) * 4 + s] = cosx[:, :, px2 + sx].transpose(0, 2, 1)
        cya[:, :, np.arange(NF) * 4 + s] = cosy[:, :, px2 + sy].transpose(0, 2, 1)

    # --- mask/depth patchify: mdf[b, 0:4, r]=mask_s, [4:8]=depth_s (raw) ---
    def patch(a):  # (B,1,H,W) -> (B, 4, R) with s=(sy*2+sx), r=py*64+px
        a6 = a[:, 0].reshape(B, PH, S, PW, S)
        return a6.transpose(0, 2, 4, 1, 3).reshape(B, 4, R).astype(f32)

    f16 = np.float16
    # mdf rows: 0..3 mask_s, 4..7 depth_s (raw), 8 ones (bias-1 feature)
    mdf = np.concatenate(
        [patch(mask), patch(depth), np.ones((B, 1, R), dtype=f32)],
        axis=1).astype(f16)                                    # (B, 9, R)

    # --- weight reshuffles ---
    w1e = weight1[:, G * 4:(G + 2 * NF) * 4].reshape(HID, 2 * NF, 4)
    w1a = np.concatenate([
        w1e[:, 0::2, :].transpose(1, 2, 0).reshape(64, HID),   # even ch (cos_x)
        w1e[:, 1::2, :].transpose(1, 2, 0).reshape(64, HID),   # odd ch (cos_y)
    ], axis=0).astype(f16)                                     # (128, HID)
    # w1bd: rows 0..3 depth weights, row 4 = bias1
    w1bd = np.concatenate(
        [weight1[:, (G + 2 * NF) * 4:].T, bias1[None, :]], axis=0).astype(f32)
    w1g = weight1[:, :G * 4].reshape(HID, G, 4).transpose(1, 2, 0)  # (256,4,HID)
    w1g = w1g.reshape(2, 128, 4 * HID).transpose(1, 0, 2).reshape(128, 2 * 4 * HID)
    w1g = np.ascontiguousarray(w1g, dtype=f16)
    w2s = weight2.T.reshape(KT2, 128, BASE).transpose(1, 0, 2).reshape(128, KT2 * BASE)
    w2s = np.ascontiguousarray(w2s, dtype=f16)
    b2t = np.ascontiguousarray(bias2.reshape(NT2, 128).T, dtype=f32)

    # stacked one-hot rhs: rows 0..63 delta(p, px), rows 64..71 delta(p, py%8)
    ohxy = np.zeros((72, CH), dtype=f16)
    ohxy[0:64] = np.tile(np.eye(64, dtype=f16), (1, CH // 64))
    ohxy[64:72] = np.repeat(np.eye(8, dtype=f16), 64, axis=1)
    selm = np.tile(np.eye(4, dtype=f16), (1, 32))           # delta(s, k%4)

    in_maps = []
    for core in range(NCORES):
        bs = [core * BPC + i for i in range(BPC)]
        # block-diagonal cos lhsT per (batch, chunk): cols 0..63 are the 64
        # cos_x features (rows = px slots), cols 64..127 the cos_y features
        # (rows 64..71 = the chunk's 8 py slots)
        cxcy = np.zeros((72, BPC * NCH * 128), dtype=f16)
        for i, b in enumerate(bs):
            for cc in range(NCH):
                c0 = (i * NCH + cc) * 128
                cxcy[0:64, c0:c0 + 64] = cxa[b]
                cxcy[64:72, c0 + 64:c0 + 128] = cya[b, 8 * cc:8 * cc + 8, :]
        gest2 = np.empty((128, 2 * BPC), dtype=f16)
        for kt in range(2):
            for i, b in enumerate(bs):
                gest2[:, kt * BPC + i] = gestalt[b, kt * 128:(kt + 1) * 128]
        in_maps.append({
            "mdf": np.ascontiguousarray(mdf[bs]),
            "cxcy": cxcy, "ohxy": ohxy, "selm": selm,
            "w1a": w1a, "w1bd": w1bd, "w1g": w1g, "gest2": gest2,
            "w2s": w2s, "b2t": b2t,
        })
    return in_maps


last_results = None  # BassKernelResults of the most recent run (for test.py)


def _enable_ldw_opt():
    """Flip walrus's --enable-ldw-opt to true so LDWEIGHTS can target the
    PE background weight buffer (weight-load / matmul overlap)."""
    import concourse.bass_utils as bu
    if getattr(bu, "_ldw_opt_patched", False):
        return
    orig = bu.run_command

    def patched(cmd, *a, **kw):
        if isinstance(cmd, list):
            cmd = ["--enable-ldw-opt=true" if c == "--enable-ldw-opt=false"
                   else c for c in cmd]
        return orig(cmd, *a, **kw)

    bu.run_command = patched
    bu._ldw_opt_patched = True


def kernel(position, gestalt, mask, depth, weight1, bias1, weight2, bias2,
           trace=False):
    global last_results
    from concourse.bass_utils import run_bass_kernel_spmd

    if "nc" not in _cache:
        _cache["nc"] = _build_nc()
    nc = _cache["nc"]

    in_maps = _host_prep(position, gestalt, mask, depth,
                         np.asarray(weight1), np.asarray(bias1),
                         np.asarray(weight2), np.asarray(bias2))
    res = run_bass_kernel_spmd(nc, in_maps, list(range(NCORES)), trace=trace)
    last_results = res

    out = np.empty((B, BASE, PH, PW), dtype=np.float32)
    for core in range(NCORES):
        o = res.results[core]["out"].reshape(BPC, BASE, PH, PW)
        out[core * BPC:(core + 1) * BPC] = o
    return out
